# revision 1
# baseline (speedup 1.0000x reference)
"""DCell hierarchy kernel for 8 Trainium2 NeuronCores.

Strategy (term/expert-parallel): each core owns 1/8 of the terms of strata
3/2/1 (256/64/16 terms).  Activations live on-chip in a "quad tile" layout:
an SBUF/PSUM tile [128, B=256] holds 4 terms, term j at partitions
32j..32j+20 (gap rows are exact zeros), batch on the free axis.  With FAN=4
this makes the children of every next-stratum term one contiguous K=128 tile,
so each per-term Linear is a single fp32 matmul; gene contributions are added
with bf16 hi/lo matmul pairs (genes are 0/1 so bf16 is exact; hi+lo recovers
fp32 weight precision).  BatchNorm batch-stats are computed per-tile with
bn_stats/bn_aggr (free-axis reduction).  The root term needs all 128 stratum-1
outputs, so each core computes its partial root pre-BN activation and a 20KB
AllReduce combines them; the root BN/tanh/head is then computed redundantly on
every core and core 0's output is used.
"""
import sys
sys.path.insert(0, '/opt/trn_rl_repo')

import numpy as np
import ml_dtypes

import concourse.bass as bass
import concourse.bacc as bacc
import concourse.mybir as mybir
from concourse import tile
from concourse.bass_utils import run_bass_kernel_spmd

F32 = mybir.dt.float32
BF16 = mybir.dt.bfloat16
AF = mybir.ActivationFunctionType

B, G, D = 256, 64, 20
T3, T2, T1 = 2048, 512, 128
FAN, EPS, NCORES = 4, 1e-5, 8
L3, L2, L1 = T3 // NCORES, T2 // NCORES, T1 // NCORES   # 256, 64, 16
Q3, Q2, Q1 = L3 // 4, L2 // 4, L1 // 4                  # 64, 16, 4
CHUNK = 8                                               # quads per BN batch

_bf16 = ml_dtypes.bfloat16


# --------------------------------------------------------------------------
# device program
# --------------------------------------------------------------------------

def _build_program():
    nc = bacc.Bacc(None, target_bir_lowering=False, debug=False)

    gt3_d = nc.dram_tensor("gt3", [16, 128, 16 * B], BF16, kind="ExternalInput")
    gt2_d = nc.dram_tensor("gt2", [4, 128, 16 * B], BF16, kind="ExternalInput")
    gt1_d = nc.dram_tensor("gt1", [128, L1 * B], BF16, kind="ExternalInput")
    gt0_d = nc.dram_tensor("gt0", [128, B], BF16, kind="ExternalInput")
    w3_d = nc.dram_tensor("w3", [128, L3 * 32], BF16, kind="ExternalInput")
    w2c_d = nc.dram_tensor("w2c", [128, L2 * 32], F32, kind="ExternalInput")
    w2g_d = nc.dram_tensor("w2g", [128, L2 * 32], BF16, kind="ExternalInput")
    w1c_d = nc.dram_tensor("w1c", [128, L1 * 32], F32, kind="ExternalInput")
    w1g_d = nc.dram_tensor("w1g", [128, L1 * 32], BF16, kind="ExternalInput")
    w0c_d = nc.dram_tensor("w0c", [128, Q1 * 20], F32, kind="ExternalInput")
    w0g_d = nc.dram_tensor("w0g", [128, 20], BF16, kind="ExternalInput")
    g3_d = nc.dram_tensor("g3b", [128, Q3], F32, kind="ExternalInput")
    be3_d = nc.dram_tensor("be3b", [128, Q3], F32, kind="ExternalInput")
    g2_d = nc.dram_tensor("g2b", [128, Q2], F32, kind="ExternalInput")
    be2_d = nc.dram_tensor("be2b", [128, Q2], F32, kind="ExternalInput")
    g1_d = nc.dram_tensor("g1b", [128, Q1], F32, kind="ExternalInput")
    be1_d = nc.dram_tensor("be1b", [128, Q1], F32, kind="ExternalInput")
    g0_d = nc.dram_tensor("g0c", [20, 1], F32, kind="ExternalInput")
    be0_d = nc.dram_tensor("be0c", [20, 1], F32, kind="ExternalInput")
    hw0_d = nc.dram_tensor("hw0c", [20, 1], F32, kind="ExternalInput")
    hb0_d = nc.dram_tensor("hb0c", [1, 1], F32, kind="ExternalInput")
    out_d = nc.dram_tensor("out", [1, B], F32, kind="ExternalOutput")

    with tile.TileContext(nc) as tc:
        with tc.tile_pool(name="const", bufs=1) as cp, \
             tc.tile_pool(name="gin", bufs=5) as gp, \
             tc.tile_pool(name="hbuf", bufs=1) as hp, \
             tc.tile_pool(name="stat", bufs=1) as sp, \
             tc.tile_pool(name="zps", bufs=8, space="PSUM") as zp, \
             tc.tile_pool(name="dram", bufs=1, space="DRAM") as dp:

            # ---- stratum-3-critical tensors first so PE starts ASAP; the
            # rest of the weights are DMA'd behind the gt3 stream ----
            w3 = cp.tile([128, L3 * 32], BF16)
            nc.sync.dma_start(out=w3[:], in_=w3_d[:])
            gb = {}
            q = Q3
            gamma3 = cp.tile([128, Q3], F32)
            nc.sync.dma_start(out=gamma3[:], in_=g3_d[:])
            beta3 = cp.tile([128, Q3], F32)
            nc.sync.dma_start(out=beta3[:], in_=be3_d[:])
            gb[3] = (gamma3, beta3)

            # ---- activation + stat buffers ----
            h3b = hp.tile([128, Q3 * B], F32)
            h2b = hp.tile([128, Q2 * B], F32)
            h1b = hp.tile([128, Q1 * B], F32)
            hbuf = {3: h3b, 2: h2b, 1: h1b}
            stats = {}
            for s, q in ((3, Q3), (2, Q2), (1, Q1)):
                stats[s] = dict(
                    st=sp.tile([128, 6 * q], F32, name=f"st{s}"),
                    mv=sp.tile([128, 2 * q], F32, name=f"mv{s}"),
                    inv=sp.tile([128, q], F32, name=f"inv{s}"),
                    sc=sp.tile([128, q], F32, name=f"sc{s}"),
                    tmp=sp.tile([128, q], F32, name=f"tmp{s}"),
                    nt=sp.tile([128, q], F32, name=f"nt{s}"),
                    bi=sp.tile([128, q], F32, name=f"bi{s}"),
                )

            def bn_smalls(s, q0, n):
                """Batched scale/bias computation for quads q0..q0+n of stratum s."""
                S = stats[s]
                gam, bet = gb[s]
                var_v = S['mv'][:, 2 * q0 + 1: 2 * (q0 + n): 2]
                mean_v = S['mv'][:, 2 * q0: 2 * (q0 + n): 2]
                # rsqrt(var+eps) entirely on DVE (magic-constant seed + 3
                # Newton steps, ~1e-7): keeps Sqrt off ACT so the tanh table
                # is loaded once for the whole kernel instead of per chunk.
                inv_v = S['inv'][:, q0:q0 + n]
                tmp_v = S['tmp'][:, q0:q0 + n]
                nc.vector.tensor_scalar(tmp_v, var_v, EPS, None,
                                        op0=mybir.AluOpType.add)
                iv = inv_v.bitcast(mybir.dt.int32)
                nc.vector.tensor_scalar(iv, tmp_v.bitcast(mybir.dt.int32), 1,
                                        -1, op0=mybir.AluOpType.arith_shift_right,
                                        op1=mybir.AluOpType.bitwise_xor)
                nc.vector.tensor_scalar(iv, iv, 0x5f3759e0, None,
                                        op0=mybir.AluOpType.add)
                nt_v = S['nt'][:, q0:q0 + n]
                for _ in range(2):
                    nc.vector.tensor_mul(nt_v, inv_v, inv_v)
                    nc.vector.tensor_mul(nt_v, nt_v, tmp_v)
                    nc.vector.tensor_scalar(nt_v, nt_v, -0.5, 1.5,
                                            op0=mybir.AluOpType.mult,
                                            op1=mybir.AluOpType.add)
                    nc.vector.tensor_mul(inv_v, inv_v, nt_v)
                nc.vector.tensor_mul(S['sc'][:, q0:q0 + n], S['inv'][:, q0:q0 + n],
                                     gam[:, q0:q0 + n])
                nc.vector.tensor_mul(S['tmp'][:, q0:q0 + n], mean_v,
                                     S['sc'][:, q0:q0 + n])
                nc.vector.tensor_sub(S['bi'][:, q0:q0 + n], bet[:, q0:q0 + n],
                                     S['tmp'][:, q0:q0 + n])

            def bn_tail(s, zt, q):
                """Per-quad stats from PSUM tile zt."""
                S = stats[s]
                nc.vector.bn_stats(S['st'][:, 6 * q:6 * q + 6], zt[:])
                nc.vector.bn_aggr(S['mv'][:, 2 * q:2 * q + 2],
                                  S['st'][:, 6 * q:6 * q + 6])

            def bn_apply(s, zt, q):
                S = stats[s]
                nc.scalar.activation(hbuf[s][:, B * q:B * (q + 1)], zt[:], AF.Tanh,
                                     bias=S['bi'][:, q:q + 1],
                                     scale=S['sc'][:, q:q + 1])

            # ================= stratum 3 =================
            # genes tiles carry each term twice on the partition axis
            # ([x; x], K=128) so one matmul applies the stacked [W_hi; W_lo]
            # weights -- fp32-precision z in a single pass per term.
            for c in range(64 // CHUNK):
                pend = []
                for gg in range(CHUNK // 4):
                    g = c * (CHUNK // 4) + gg
                    gt3 = gp.tile([128, 16 * B], BF16, name="gt3t", tag="gt3t")
                    nc.sync.dma_start(out=gt3[:], in_=gt3_d[g, :, :])
                    for qq in range(4):
                        q = g * 4 + qq
                        # two quads share one PSUM bank (free-axis halves) so
                        # 8 banks hold 2 chunks and chunk c+1's matmuls overlap
                        # chunk c's BN tail
                        if qq % 2 == 0:
                            zpair = zp.tile([128, 2 * B], F32, name="z3t", tag="z")
                        zt = zpair[:, B * (qq % 2):B * (qq % 2 + 1)]
                        for j in range(4):
                            t = 4 * q + j
                            slot = t - 16 * g
                            rhs = gt3[:, B * slot:B * (slot + 1)]
                            nc.tensor.matmul(zt[32 * j:32 * j + 32, :],
                                             w3[:, 32 * t:32 * t + 32], rhs,
                                             start=True, stop=True,
                                             tile_position=(0, 32 * j))
                        bn_tail(3, zt, q)
                        pend.append((zt, q))
                bn_smalls(3, c * CHUNK, CHUNK)
                for zt, q in pend:
                    bn_apply(3, zt, q)

            # ---- stratum-2 weights (DMA'd during stratum-3 compute) ----
            w2c = cp.tile([128, L2 * 32], F32)
            nc.sync.dma_start(out=w2c[:], in_=w2c_d[:])
            w2g = cp.tile([128, L2 * 32], BF16)
            nc.sync.dma_start(out=w2g[:], in_=w2g_d[:])
            gamma2 = cp.tile([128, Q2], F32)
            nc.sync.dma_start(out=gamma2[:], in_=g2_d[:])
            beta2 = cp.tile([128, Q2], F32)
            nc.sync.dma_start(out=beta2[:], in_=be2_d[:])
            gb[2] = (gamma2, beta2)

            # ================= strata 2 and 1 =================
            def mid_stratum(s, nq, wc, wg, gtiles, pair_cols):
                """s: stratum id; nq: #quads; wc/wg: weights; gtiles(q)->(tile, pig)"""
                prev = hbuf[s + 1]
                for c0 in range(0, nq, CHUNK):
                    nch = min(CHUNK, nq - c0)
                    pend = []
                    for qq in range(nch):
                        q = c0 + qq
                        if qq % 2 == 0:
                            zpair = zp.tile([128, 2 * B], F32, name=f"z{s}t",
                                            tag="z")
                        zt = zpair[:, B * (qq % 2):B * (qq % 2 + 1)]
                        for j in range(4):
                            u = 4 * q + j
                            # children: K=128 fp32 matmul over the quad tile u,
                            # then the term's bf16 hi/lo gene matmuls close the
                            # accumulation group before the next term opens one
                            # (interleaved open groups in a bank are illegal).
                            nc.tensor.matmul(
                                zt[32 * j:32 * j + 32, :],
                                wc[:, 32 * u:32 * u + 32],
                                prev[:, B * u:B * (u + 1)],
                                start=True, stop=False, tile_position=(0, 32 * j))
                            gt_, slot = gtiles(u)
                            rhs = gt_[:, B * slot:B * (slot + 1)]
                            nc.tensor.matmul(zt[32 * j:32 * j + 32, :],
                                             wg[:, 32 * u:32 * u + 32], rhs,
                                             start=False, stop=True,
                                             tile_position=(0, 32 * j))
                        bn_tail(s, zt, q)
                        pend.append((zt, q))
                    bn_smalls(s, c0, nch)
                    for zt, q in pend:
                        bn_apply(s, zt, q)

            # stratum 2: four genes groups of 16 terms
            g2tiles = []
            for grp in range(4):
                g2t = gp.tile([128, 16 * B], BF16, name="gt2t", tag="gt2t", bufs=4)
                nc.sync.dma_start(out=g2t[:], in_=gt2_d[grp, :, :])
                g2tiles.append(g2t)

            def gt2_lookup(u):
                return g2tiles[u // 16], u % 16

            # ---- stratum-1 + root weights (DMA'd during stratum-3/2) ----
            w1c = cp.tile([128, L1 * 32], F32)
            nc.sync.dma_start(out=w1c[:], in_=w1c_d[:])
            w1g = cp.tile([128, L1 * 32], BF16)
            nc.sync.dma_start(out=w1g[:], in_=w1g_d[:])
            gt1 = cp.tile([128, L1 * B], BF16)
            nc.sync.dma_start(out=gt1[:], in_=gt1_d[:])
            gamma1 = cp.tile([128, Q1], F32)
            nc.sync.dma_start(out=gamma1[:], in_=g1_d[:])
            beta1 = cp.tile([128, Q1], F32)
            nc.sync.dma_start(out=beta1[:], in_=be1_d[:])
            gb[1] = (gamma1, beta1)
            w0c = cp.tile([128, Q1 * 20], F32)
            nc.sync.dma_start(out=w0c[:], in_=w0c_d[:])
            w0g = cp.tile([128, 20], BF16)
            nc.sync.dma_start(out=w0g[:], in_=w0g_d[:])
            gt0 = cp.tile([128, B], BF16)
            nc.sync.dma_start(out=gt0[:], in_=gt0_d[:])
            g0c = cp.tile([20, 1], F32)
            nc.sync.dma_start(out=g0c[:], in_=g0_d[:])
            be0c = cp.tile([20, 1], F32)
            nc.sync.dma_start(out=be0c[:], in_=be0_d[:])
            hw0 = cp.tile([20, 1], F32)
            nc.sync.dma_start(out=hw0[:], in_=hw0_d[:])
            hb0 = cp.tile([1, 1], F32)
            nc.sync.dma_start(out=hb0[:], in_=hb0_d[:])

            mid_stratum(2, Q2, w2c, w2g, gt2_lookup, None)

            def gt1_lookup(u):
                return gt1, u

            mid_stratum(1, Q1, w1c, w1g, gt1_lookup, None)

            # ================= root =================
            zr = zp.tile([20, B], F32, name="zr", tag="z")
            for q1 in range(Q1):
                nc.tensor.matmul(zr[:], w0c[:, 20 * q1:20 * (q1 + 1)],
                                 h1b[:, B * q1:B * (q1 + 1)],
                                 start=(q1 == 0), stop=False)
            nc.tensor.matmul(zr[:], w0g[:], gt0[:], start=False, stop=True)

            z0p = sp.tile([20, B], F32)
            nc.vector.tensor_copy(z0p[:], zr[:])

            cc_in = dp.tile([20, B], F32)
            cc_out = dp.tile([20, B], F32, addr_space="Shared")
            nc.gpsimd.dma_start(out=cc_in[:], in_=z0p[:])
            nc.gpsimd.collective_compute(
                "AllReduce", mybir.AluOpType.add,
                replica_groups=[list(range(NCORES))],
                ins=[cc_in.opt()], outs=[cc_out.opt()])
            z0 = sp.tile([20, B], F32)
            nc.gpsimd.dma_start(out=z0[:], in_=cc_out[:])

            st0 = sp.tile([20, 6], F32)
            nc.vector.bn_stats(st0[:], z0[:])
            mv0 = sp.tile([20, 2], F32)
            nc.vector.bn_aggr(mv0[:], st0[:])
            inv0 = sp.tile([20, 1], F32)
            sd0 = sp.tile([20, 1], F32)
            nt0 = sp.tile([20, 1], F32)
            nc.vector.tensor_scalar(sd0[:], mv0[:, 1:2], EPS, None,
                                    op0=mybir.AluOpType.add)
            iv0 = inv0[:].bitcast(mybir.dt.int32)
            nc.vector.tensor_scalar(iv0, sd0[:].bitcast(mybir.dt.int32), 1,
                                    -1, op0=mybir.AluOpType.arith_shift_right,
                                    op1=mybir.AluOpType.bitwise_xor)
            nc.vector.tensor_scalar(iv0, iv0, 0x5f3759e0, None,
                                    op0=mybir.AluOpType.add)
            for _ in range(2):
                nc.vector.tensor_mul(nt0[:], inv0[:], inv0[:])
                nc.vector.tensor_mul(nt0[:], nt0[:], sd0[:])
                nc.vector.tensor_scalar(nt0[:], nt0[:], -0.5, 1.5,
                                        op0=mybir.AluOpType.mult,
                                        op1=mybir.AluOpType.add)
                nc.vector.tensor_mul(inv0[:], inv0[:], nt0[:])
            sc0 = sp.tile([20, 1], F32)
            nc.vector.tensor_mul(sc0[:], inv0[:], g0c[:])
            tmp0 = sp.tile([20, 1], F32)
            nc.vector.tensor_mul(tmp0[:], mv0[:, 0:1], sc0[:])
            bi0 = sp.tile([20, 1], F32)
            nc.vector.tensor_sub(bi0[:], be0c[:], tmp0[:])
            h0 = sp.tile([20, B], F32)
            nc.scalar.activation(h0[:], z0[:], AF.Tanh, bias=bi0[:], scale=sc0[:])

            zh = zp.tile([1, B], F32, name="zh", tag="z")
            nc.tensor.matmul(zh[:], hw0[:], h0[:], start=True, stop=True)
            osb = sp.tile([1, B], F32)
            nc.scalar.activation(osb[:], zh[:], AF.Identity,
                                 bias=hb0[:], scale=1.0)
            nc.sync.dma_start(out=out_d[:], in_=osb[:])

    nc.compile()
    return nc


_PROGRAM = None


def _program():
    global _PROGRAM
    if _PROGRAM is None:
        _PROGRAM = _build_program()
    return _PROGRAM


# --------------------------------------------------------------------------
# host-side sharding / layout
# --------------------------------------------------------------------------

def _genes_tiles(genes_slice):
    """[B, T, G] fp32 -> duplicated term tiles [ngrp, 128, 16*B] bf16.

    Each term tile is [x; x] (the term's G=64 gene rows stacked twice) so a
    single matmul against stacked [W_hi; W_lo] weights gives hi+lo in one
    pass."""
    t = genes_slice.shape[1]
    x = np.ascontiguousarray(genes_slice.transpose(1, 2, 0))      # [T, G, B]
    x = np.concatenate([x, x], axis=1)                            # [T, 128, B]
    if t >= 16:
        x = x.reshape(t // 16, 16, 128, B).transpose(0, 2, 1, 3)
        x = np.ascontiguousarray(x).reshape(t // 16, 128, 16 * B)
    else:
        x = np.ascontiguousarray(x.transpose(1, 0, 2)).reshape(1, 128, t * B)
    return x.astype(_bf16)


def _hilo(w):
    hi = w.astype(_bf16)
    lo = (w - hi.astype(np.float32)).astype(_bf16)
    return hi, lo


def _w_leaf(w3_slice):
    """[L3, G, D] -> [128, L3*32] bf16: per term [W_hi; W_lo] stacked on K."""
    L = w3_slice.shape[0]
    wp = np.zeros((L, 64, 32), np.float32)
    wp[:, :, :D] = w3_slice
    hi, lo = _hilo(wp)
    hl = np.concatenate([hi.astype(np.float32), lo.astype(np.float32)], axis=1)
    arr = hl.transpose(1, 0, 2)                                   # [128, L, 32]
    return np.ascontiguousarray(arr).reshape(128, L * 32).astype(_bf16)


def _w_children(w_slice):
    """[L, 144, D] -> gappy [128, L*32] fp32 from children rows 0:80."""
    L = w_slice.shape[0]
    ch = w_slice[:, :80, :].reshape(L, 4, 20, D)
    out = np.zeros((L, 4, 32, 32), np.float32)
    out[:, :, :20, :D] = ch
    out = out.reshape(L, 128, 32).transpose(1, 0, 2)
    return np.ascontiguousarray(out).reshape(128, L * 32)


def _w_genes(w_slice):
    """[L, 144, D] gene rows 80:144 -> [128, L*32] bf16 [W_hi; W_lo] stacked."""
    L = w_slice.shape[0]
    wp = np.zeros((L, 64, 32), np.float32)
    wp[:, :, :D] = w_slice[:, 80:144, :]
    hi, lo = _hilo(wp)
    hl = np.concatenate([hi.astype(np.float32), lo.astype(np.float32)], axis=1)
    arr = hl.transpose(1, 0, 2)
    return np.ascontiguousarray(arr).reshape(128, L * 32).astype(_bf16)


def _gappy_cols(vec_slice):
    """[L, D] -> [128, L/4] with row 32j+d, col q = vec[4q+j, d]; gaps zero."""
    L = vec_slice.shape[0]
    arr = vec_slice.reshape(L // 4, 4, D)
    out = np.zeros((L // 4, 4, 32), np.float32)
    out[:, :, :D] = arr
    out = out.reshape(L // 4, 128).T
    return np.ascontiguousarray(out)


def _prep_core(c, iv):
    s3 = slice(L3 * c, L3 * (c + 1))
    s2 = slice(L2 * c, L2 * (c + 1))
    s1 = slice(L1 * c, L1 * (c + 1))

    w0 = iv['W0'][0]                                    # [2624, 20]
    w0h = w0[:T1 * D, :].reshape(T1, D, D)[L1 * c:L1 * (c + 1)]   # [16, 20, 20]
    arr = w0h.reshape(Q1, 4, 20, D)
    w0c = np.zeros((Q1, 4, 32, D), np.float32)
    w0c[:, :, :20, :] = arr
    w0c = w0c.reshape(Q1, 128, D).transpose(1, 0, 2)
    w0c = np.ascontiguousarray(w0c).reshape(128, Q1 * D)

    w0g_hi, w0g_lo = _hilo((w0[T1 * D:, :] / NCORES).astype(np.float32))
    w0g = np.concatenate([w0g_hi.astype(np.float32),
                          w0g_lo.astype(np.float32)], axis=0).astype(_bf16)

    return {
        'gt3': _genes_tiles(iv['genes3'][:, s3, :]),
        'gt2': _genes_tiles(iv['genes2'][:, s2, :]),
        'gt1': _genes_tiles(iv['genes1'][:, s1, :])[0],
        'gt0': np.ascontiguousarray(
            np.concatenate([iv['genes0'][:, 0, :].T] * 2, axis=0)).astype(_bf16),
        'w3': _w_leaf(iv['W3'][s3]),
        'w2c': _w_children(iv['W2'][s2]),
        'w2g': _w_genes(iv['W2'][s2]),
        'w1c': _w_children(iv['W1'][s1]),
        'w1g': _w_genes(iv['W1'][s1]),
        'w0c': w0c,
        'w0g': w0g,
        'g3b': _gappy_cols(iv['g3'][s3]), 'be3b': _gappy_cols(iv['be3'][s3]),
        'g2b': _gappy_cols(iv['g2'][s2]), 'be2b': _gappy_cols(iv['be2'][s2]),
        'g1b': _gappy_cols(iv['g1'][s1]), 'be1b': _gappy_cols(iv['be1'][s1]),
        'g0c': np.ascontiguousarray(iv['g0'].reshape(1, D).T),
        'be0c': np.ascontiguousarray(iv['be0'].reshape(1, D).T),
        'hw0c': np.ascontiguousarray(iv['hw0'][0]),      # [20, 1]
        'hb0c': np.ascontiguousarray(iv['hb0']).reshape(1, 1),
    }


def _prep_inputs(inputs):
    iv = {k: np.asarray(v, dtype=np.float32) for k, v in inputs.items()}
    return [_prep_core(c, iv) for c in range(NCORES)]


def run(in_maps, **kwargs):
    nc = _program()
    return run_bass_kernel_spmd(nc, in_maps, core_ids=list(range(NCORES)), **kwargs)


def kernel(**inputs) -> np.ndarray:
    in_maps = _prep_inputs(inputs)
    res = run(in_maps)
    pred = np.asarray(res.results[0]['out'], dtype=np.float32)   # [1, B]
    return np.ascontiguousarray(pred.T)                          # [B, 1]



# revision 13
# speedup vs baseline: 1.0630x; 1.0630x over previous
"""DCell hierarchy kernel for 8 Trainium2 NeuronCores.

Term-parallel: each core owns 1/8 of strata 3/2/1 (256/64/16 terms).
Activations live on-chip in quad tiles [128, B=256] (term j of the quad at
partitions 32j..32j+20, batch on the free axis).

v2 vs the original baseline:
- No bf16 hi/lo weight splitting: the correctness gate is 2e-2 and single
  bf16 gene-weights land at ~1.7e-3 (measured in fp64 sim), so gene matmuls
  are 2-term block-diagonal pairs: stationary [128, 64] holds term A's
  weights on K-rows 0-63 and term B's on 64-127; the moving gene tile
  [128, B] stacks the two terms' gene states.  Halves both gene DMA and
  PE rows vs the duplicated hi/lo layout.
- Children matmuls (strata 2/1/0) and h buffers are bf16 (1 cycle/row
  instead of float32's 4; all-bf16 network measures 2.9e-3 in fp64 sim).
- BN: one bn_stats per PSUM bank covers 2 quads ([128, 2, 256] 3D AP);
  bn_aggr is gone -- mean/var come straight from the 6-stat layout
  (count/mean/M2 for even and odd elements) with chunk-batched ALU ops on
  GPSIMD, keeping DVE free for stats.
- Root head folds hb0 as a 21st K-row (ones row in h0) so no extra
  activation op after the head matmul.
"""
import sys
sys.path.insert(0, '/opt/trn_rl_repo')

import numpy as np
import ml_dtypes

import concourse.bass as bass
import concourse.bacc as bacc
import concourse.mybir as mybir
from concourse import tile
from concourse.bass_utils import run_bass_kernel_spmd

F32 = mybir.dt.float32
F32R = mybir.dt.float32r
BF16 = mybir.dt.bfloat16
AF = mybir.ActivationFunctionType
ALU = mybir.AluOpType

B, G, D = 256, 64, 20
T3, T2, T1 = 2048, 512, 128
FAN, EPS, NCORES = 4, 1e-5, 8
L3, L2, L1 = T3 // NCORES, T2 // NCORES, T1 // NCORES   # 256, 64, 16
Q3, Q2, Q1 = L3 // 4, L2 // 4, L1 // 4                  # 64, 16, 4
P3, P2, P1 = L3 // 2, L2 // 2, L1 // 2                  # 128, 32, 8 pairs
CHUNK = 8                                               # quads per BN chunk

SMALLS_ON_GPSIMD = True

_bf16 = ml_dtypes.bfloat16


# --------------------------------------------------------------------------
# device program
# --------------------------------------------------------------------------

def _build_program():
    nc = bacc.Bacc(None, target_bir_lowering=False, debug=False)

    gt3_d = nc.dram_tensor("gt3", [Q3 // CHUNK, 128, 2 * CHUNK * B], BF16,
                           kind="ExternalInput")
    gt2_d = nc.dram_tensor("gt2", [Q2 // CHUNK, 128, 2 * CHUNK * B], BF16,
                           kind="ExternalInput")
    gt1_d = nc.dram_tensor("gt1", [128, P1 * B], BF16, kind="ExternalInput")
    gt0_d = nc.dram_tensor("gt0", [64, B], BF16, kind="ExternalInput")
    w3_d = nc.dram_tensor("w3", [128, P3 * 64], BF16, kind="ExternalInput")
    w2c_d = nc.dram_tensor("w2c", [128, L2 * 32], BF16, kind="ExternalInput")
    w2g_d = nc.dram_tensor("w2g", [128, P2 * 64], BF16, kind="ExternalInput")
    w1c_d = nc.dram_tensor("w1c", [128, L1 * 32], BF16, kind="ExternalInput")
    w1g_d = nc.dram_tensor("w1g", [128, P1 * 64], BF16, kind="ExternalInput")
    w0c_d = nc.dram_tensor("w0c", [128, Q1 * 20], BF16, kind="ExternalInput")
    w0g_d = nc.dram_tensor("w0g", [64, 20], BF16, kind="ExternalInput")
    g3_d = nc.dram_tensor("g3b", [128, Q3], F32, kind="ExternalInput")
    be3_d = nc.dram_tensor("be3b", [128, Q3], F32, kind="ExternalInput")
    g2_d = nc.dram_tensor("g2b", [128, Q2], F32, kind="ExternalInput")
    be2_d = nc.dram_tensor("be2b", [128, Q2], F32, kind="ExternalInput")
    g1_d = nc.dram_tensor("g1b", [128, Q1], F32, kind="ExternalInput")
    be1_d = nc.dram_tensor("be1b", [128, Q1], F32, kind="ExternalInput")
    g0_d = nc.dram_tensor("g0c", [20, 1], F32, kind="ExternalInput")
    be0_d = nc.dram_tensor("be0c", [20, 1], F32, kind="ExternalInput")
    hw0hb_d = nc.dram_tensor("hw0hb", [33, 1], F32, kind="ExternalInput")
    out_d = nc.dram_tensor("out", [1, B], F32, kind="ExternalOutput")

    with tile.TileContext(nc) as tc:
        with tc.tile_pool(name="const", bufs=1) as cp, \
             tc.tile_pool(name="gin", bufs=3) as gp, \
             tc.tile_pool(name="hbuf", bufs=1) as hp, \
             tc.tile_pool(name="stat", bufs=1) as sp, \
             tc.tile_pool(name="zps", bufs=8, space="PSUM") as zp, \
             tc.tile_pool(name="dram", bufs=1, space="DRAM") as dp:

            # ---- stratum-3-critical tensors first so PE starts ASAP ----
            w3 = cp.tile([128, P3 * 64], BF16)
            nc.sync.dma_start(out=w3[:], in_=w3_d[:])
            gb = {}
            gamma3 = cp.tile([128, Q3], F32)
            nc.sync.dma_start(out=gamma3[:], in_=g3_d[:])
            beta3 = cp.tile([128, Q3], F32)
            nc.sync.dma_start(out=beta3[:], in_=be3_d[:])
            gb[3] = (gamma3, beta3)

            # ---- activation + stat buffers ----
            h3b = hp.tile([128, Q3 * B], BF16)
            h2b = hp.tile([128, Q2 * B], BF16)
            h1b = hp.tile([128, Q1 * B], BF16)
            hbuf = {3: h3b, 2: h2b, 1: h1b}
            stats = {}
            for s, q in ((3, Q3), (2, Q2), (1, Q1)):
                stats[s] = dict(
                    st=sp.tile([128, 6 * q], F32, name=f"st{s}"),
                    ssum=sp.tile([128, q], F32, name=f"ssum{s}"),
                    sdif=sp.tile([128, q], F32, name=f"sdif{s}"),
                    vex=sp.tile([128, q], F32, name=f"vex{s}"),
                    inv=sp.tile([128, q], F32, name=f"inv{s}"),
                    tm=sp.tile([128, q], F32, name=f"tm{s}"),
                    nt=sp.tile([128, q], F32, name=f"nt{s}"),
                    sc=sp.tile([128, q], F32, name=f"sc{s}"),
                    bi=sp.tile([128, q], F32, name=f"bi{s}"),
                )

            eng = nc.gpsimd if SMALLS_ON_GPSIMD else nc.vector

            def bn_smalls(s, q0, n):
                """Scale/bias for quads q0..q0+n of stratum s, straight from
                the 6-stat (count,mean,M2)x{even,odd} layout:
                mean = (me+mo)/2,  var = (M2e+M2o)/256 + ((me-mo)/2)^2."""
                S = stats[s]
                gam, bet = gb[s]
                st = S['st']
                me = st[:, 6 * q0 + 1: 6 * (q0 + n): 6]
                mo = st[:, 6 * q0 + 4: 6 * (q0 + n): 6]
                cve = st[:, 6 * q0 + 2: 6 * (q0 + n): 6]
                cvo = st[:, 6 * q0 + 5: 6 * (q0 + n): 6]
                sl = slice(q0, q0 + n)
                ssum, sdif = S['ssum'][:, sl], S['sdif'][:, sl]
                vex, inv = S['vex'][:, sl], S['inv'][:, sl]
                tm, nt = S['tm'][:, sl], S['nt'][:, sl]
                sc, bi = S['sc'][:, sl], S['bi'][:, sl]
                eng.tensor_tensor(ssum, me, mo, op=ALU.add)
                eng.tensor_tensor(sdif, me, mo, op=ALU.subtract)
                eng.tensor_tensor(vex, cve, cvo, op=ALU.add)
                eng.tensor_tensor(tm, sdif, sdif, op=ALU.mult)
                eng.tensor_scalar(vex, vex, 1.0 / B, None, op0=ALU.mult)
                eng.tensor_scalar(tm, tm, 0.25, EPS, op0=ALU.mult, op1=ALU.add)
                eng.tensor_tensor(vex, vex, tm, op=ALU.add)
                # rsqrt: magic-constant seed + 2 Newton steps.  The int-typed
                # seed ops only codegen on DVE; the rest stays on `eng`.
                iv = inv.bitcast(mybir.dt.int32)
                nc.vector.tensor_scalar(iv, vex.bitcast(mybir.dt.int32), 1, -1,
                                        op0=ALU.arith_shift_right,
                                        op1=ALU.bitwise_xor)
                nc.vector.tensor_scalar(iv, iv, 0x5f3759e0, None, op0=ALU.add)
                for _ in range(2):
                    eng.tensor_tensor(nt, inv, inv, op=ALU.mult)
                    eng.tensor_tensor(nt, nt, vex, op=ALU.mult)
                    eng.tensor_scalar(nt, nt, -0.5, 1.5, op0=ALU.mult,
                                      op1=ALU.add)
                    eng.tensor_tensor(inv, inv, nt, op=ALU.mult)
                eng.tensor_tensor(sc, inv, gam[:, sl], op=ALU.mult)
                eng.tensor_tensor(tm, ssum, sc, op=ALU.mult)
                eng.tensor_scalar(tm, tm, 0.5, None, op0=ALU.mult)
                eng.tensor_tensor(bi, bet[:, sl], tm, op=ALU.subtract)

            def bn_apply(s, zq, q):
                S = stats[s]
                nc.scalar.activation(hbuf[s][:, B * q:B * (q + 1)], zq, AF.Tanh,
                                     bias=S['bi'][:, q:q + 1],
                                     scale=S['sc'][:, q:q + 1])

            # ================= stratum 3 =================
            for c in range(Q3 // CHUNK):
                gt3t = gp.tile([128, 2 * CHUNK * B], BF16, name="gt3t",
                               tag="gt3t")
                nc.sync.dma_start(out=gt3t[:], in_=gt3_d[c, :, :])
                pend = []
                for qq in range(CHUNK):
                    q = c * CHUNK + qq
                    if qq % 2 == 0:
                        zpair = zp.tile([128, 2, B], F32, name="z3t", tag="z")
                    zq = zpair[:, qq % 2, :]
                    for half in range(2):
                        p = 2 * q + half            # pair index
                        slot = p - 2 * c * CHUNK    # pair slot in this tile
                        nc.tensor.matmul(zq[64 * half:64 * half + 64, :],
                                         w3[:, 64 * p:64 * p + 64],
                                         gt3t[:, B * slot:B * (slot + 1)],
                                         start=True, stop=True,
                                         tile_position=(0, 64 * half))
                    pend.append((zq, q))
                    nc.vector.bn_stats(stats[3]['st'][:, 6 * q:6 * q + 6], zq)
                bn_smalls(3, c * CHUNK, CHUNK)
                for zq, q in pend:
                    bn_apply(3, zq, q)

                if c == 1:
                    # stratum-2 weights stream in behind the gene tiles
                    w2c = cp.tile([128, L2 * 32], BF16)
                    nc.sync.dma_start(out=w2c[:], in_=w2c_d[:])
                    w2g = cp.tile([128, P2 * 64], BF16)
                    nc.sync.dma_start(out=w2g[:], in_=w2g_d[:])
                    gamma2 = cp.tile([128, Q2], F32)
                    nc.sync.dma_start(out=gamma2[:], in_=g2_d[:])
                    beta2 = cp.tile([128, Q2], F32)
                    nc.sync.dma_start(out=beta2[:], in_=be2_d[:])
                    gb[2] = (gamma2, beta2)
                if c == 3:
                    w1c = cp.tile([128, L1 * 32], BF16)
                    nc.sync.dma_start(out=w1c[:], in_=w1c_d[:])
                    w1g = cp.tile([128, P1 * 64], BF16)
                    nc.sync.dma_start(out=w1g[:], in_=w1g_d[:])
                    gt1 = cp.tile([128, P1 * B], BF16)
                    nc.sync.dma_start(out=gt1[:], in_=gt1_d[:])
                    gamma1 = cp.tile([128, Q1], F32)
                    nc.sync.dma_start(out=gamma1[:], in_=g1_d[:])
                    beta1 = cp.tile([128, Q1], F32)
                    nc.sync.dma_start(out=beta1[:], in_=be1_d[:])
                    gb[1] = (gamma1, beta1)
                    w0c = cp.tile([128, Q1 * 20], BF16)
                    nc.sync.dma_start(out=w0c[:], in_=w0c_d[:])
                    w0g = cp.tile([64, 20], BF16)
                    nc.sync.dma_start(out=w0g[:], in_=w0g_d[:])
                    gt0 = cp.tile([64, B], BF16)
                    nc.sync.dma_start(out=gt0[:], in_=gt0_d[:])
                    g0c = cp.tile([20, 1], F32)
                    nc.sync.dma_start(out=g0c[:], in_=g0_d[:])
                    be0c = cp.tile([20, 1], F32)
                    nc.sync.dma_start(out=be0c[:], in_=be0_d[:])
                    hw0hb = cp.tile([33, 1], F32)
                    nc.sync.dma_start(out=hw0hb[:], in_=hw0hb_d[:])

            # ================= strata 2 and 1 =================
            def mid_stratum(s, nq, wc, wg, gtile_lookup):
                prev = hbuf[s + 1]
                wcr = wc[:]
                prevr = prev[:]
                for c0 in range(0, nq, CHUNK):
                    nch = min(CHUNK, nq - c0)
                    pend = []
                    for qq in range(nch):
                        q = c0 + qq
                        if qq % 2 == 0:
                            zpair = zp.tile([128, 2, B], F32, name=f"z{s}t",
                                            tag="z")
                        zq = zpair[:, qq % 2, :]
                        # gene pair matmuls open the bank (their zero weight
                        # rows also zero the gap partitions), children
                        # accumulate on top as float32r.
                        for half in range(2):
                            p = 2 * q + half
                            gt_, slot = gtile_lookup(p)
                            nc.tensor.matmul(zq[64 * half:64 * half + 64, :],
                                             wg[:, 64 * p:64 * p + 64],
                                             gt_[:, B * slot:B * (slot + 1)],
                                             start=True, stop=False,
                                             tile_position=(0, 64 * half),
                                             skip_group_check=True)
                        for j in range(4):
                            u = 4 * q + j
                            nc.tensor.matmul(
                                zq[32 * j:32 * j + 32, :],
                                wcr[:, 32 * u:32 * u + 32],
                                prevr[:, B * u:B * (u + 1)],
                                start=False, stop=True,
                                tile_position=(0, 32 * j),
                                skip_group_check=True)
                        pend.append((zq, q))
                        nc.vector.bn_stats(stats[s]['st'][:, 6 * q:6 * q + 6],
                                           zq)
                    bn_smalls(s, c0, nch)
                    for zq, q in pend:
                        bn_apply(s, zq, q)

            g2tiles = []
            for grp in range(Q2 // CHUNK):
                g2t = gp.tile([128, 2 * CHUNK * B], BF16, name="gt2t",
                              tag="gt2t", bufs=2)
                nc.sync.dma_start(out=g2t[:], in_=gt2_d[grp, :, :])
                g2tiles.append(g2t)

            mid_stratum(2, Q2, w2c, w2g,
                        lambda p: (g2tiles[p // (2 * CHUNK)],
                                   p % (2 * CHUNK)))
            mid_stratum(1, Q1, w1c, w1g, lambda p: (gt1, p))

            # ================= root =================
            zr = zp.tile([20, B], F32, name="zr", tag="z")
            w0cr = w0c[:]
            h1r = h1b[:]
            for q1 in range(Q1):
                nc.tensor.matmul(zr[:], w0cr[:, 20 * q1:20 * (q1 + 1)],
                                 h1r[:, B * q1:B * (q1 + 1)],
                                 start=(q1 == 0), stop=False)
            nc.tensor.matmul(zr[:], w0g[:], gt0[:], start=False, stop=True)

            z0p = sp.tile([20, B], F32)
            nc.vector.tensor_copy(z0p[:], zr[:])

            cc_in = dp.tile([20, B], F32)
            cc_out = dp.tile([20, B], F32, addr_space="Shared")
            nc.gpsimd.dma_start(out=cc_in[:], in_=z0p[:])
            nc.gpsimd.collective_compute(
                "AllReduce", ALU.add,
                replica_groups=[list(range(NCORES))],
                ins=[cc_in.opt()], outs=[cc_out.opt()])
            z0 = sp.tile([20, B], F32)
            nc.gpsimd.dma_start(out=z0[:], in_=cc_out[:])

            # root BN: single bn_stats, mean/var from the 6-stat layout
            st0 = sp.tile([20, 6], F32)
            nc.vector.bn_stats(st0[:], z0[:])
            me0, mo0 = st0[:, 1:2], st0[:, 4:5]
            cve0, cvo0 = st0[:, 2:3], st0[:, 5:6]
            s0 = sp.tile([20, 1], F32)
            d0 = sp.tile([20, 1], F32)
            v0 = sp.tile([20, 1], F32)
            i0 = sp.tile([20, 1], F32)
            t0 = sp.tile([20, 1], F32)
            n0 = sp.tile([20, 1], F32)
            V = nc.vector
            V.tensor_tensor(s0[:], me0, mo0, op=ALU.add)
            V.tensor_tensor(d0[:], me0, mo0, op=ALU.subtract)
            V.tensor_tensor(v0[:], cve0, cvo0, op=ALU.add)
            V.tensor_tensor(t0[:], d0[:], d0[:], op=ALU.mult)
            V.tensor_scalar(v0[:], v0[:], 1.0 / B, None, op0=ALU.mult)
            V.tensor_scalar(t0[:], t0[:], 0.25, EPS, op0=ALU.mult, op1=ALU.add)
            V.tensor_tensor(v0[:], v0[:], t0[:], op=ALU.add)
            iv0 = i0[:].bitcast(mybir.dt.int32)
            V.tensor_scalar(iv0, v0[:].bitcast(mybir.dt.int32), 1, -1,
                            op0=ALU.arith_shift_right, op1=ALU.bitwise_xor)
            V.tensor_scalar(iv0, iv0, 0x5f3759e0, None, op0=ALU.add)
            for _ in range(2):
                V.tensor_tensor(n0[:], i0[:], i0[:], op=ALU.mult)
                V.tensor_tensor(n0[:], n0[:], v0[:], op=ALU.mult)
                V.tensor_scalar(n0[:], n0[:], -0.5, 1.5, op0=ALU.mult,
                                op1=ALU.add)
                V.tensor_tensor(i0[:], i0[:], n0[:], op=ALU.mult)
            sc0 = sp.tile([20, 1], F32)
            V.tensor_tensor(sc0[:], i0[:], g0c[:], op=ALU.mult)
            V.tensor_tensor(t0[:], s0[:], sc0[:], op=ALU.mult)
            V.tensor_scalar(t0[:], t0[:], 0.5, None, op0=ALU.mult)
            bi0 = sp.tile([20, 1], F32)
            V.tensor_tensor(bi0[:], be0c[:], t0[:], op=ALU.subtract)

            # h0 with a ones row so the head matmul folds hb0; the row sits
            # at partition 32 (engine partition bases must be 32-aligned),
            # rows 20..31 are zeroed once so the matmul reads no garbage.
            h0 = sp.tile([33, B], F32)
            nc.vector.memset(h0[0:33, :], 0.0)
            nc.vector.memset(h0[32:33, :], 1.0)
            nc.scalar.activation(h0[0:20, :], z0[:], AF.Tanh,
                                 bias=bi0[:], scale=sc0[:])
            zh = zp.tile([1, B], F32, name="zh", tag="z")
            nc.tensor.matmul(zh[:], hw0hb[:], h0[:], start=True, stop=True)
            osb = sp.tile([1, B], F32)
            nc.vector.tensor_copy(osb[:], zh[:])
            nc.sync.dma_start(out=out_d[:], in_=osb[:])

    nc.compile()
    return nc


_PROGRAM = None


def _program():
    global _PROGRAM
    if _PROGRAM is None:
        _PROGRAM = _build_program()
    return _PROGRAM


# --------------------------------------------------------------------------
# host-side sharding / layout
# --------------------------------------------------------------------------

def _genes_pairs(genes_slice, group):
    """[B, T, G] fp32 -> pair tiles: [T//(2*group), 128, group*B] bf16.

    Pair p stacks term 2p's genes on K-rows 0-63 and term 2p+1's on 64-127.
    `group` pairs are packed per DMA tile."""
    t = genes_slice.shape[1]
    x = np.ascontiguousarray(genes_slice.transpose(1, 2, 0))      # [T, G, B]
    x = x.reshape(t // 2, 128, B)                                  # pairs
    p = t // 2
    if group > 1:
        x = x.reshape(p // group, group, 128, B).transpose(0, 2, 1, 3)
        x = np.ascontiguousarray(x).reshape(p // group, 128, group * B)
    else:
        x = x.reshape(1, 128, p * B) if p > 1 else x.reshape(1, 128, B)
    return x.astype(_bf16)


def _w_pairs(w_slice):
    """[L, 64, D] gene weights -> [128, (L/2)*64] bf16 block-diag pairs."""
    L = w_slice.shape[0]
    out = np.zeros((L // 2, 128, 64), np.float32)
    out[:, 0:64, 0:D] = w_slice[0::2]
    out[:, 64:128, 32:32 + D] = w_slice[1::2]
    out = out.transpose(1, 0, 2)
    return np.ascontiguousarray(out).reshape(128, (L // 2) * 64).astype(_bf16)


def _w_children(w_slice):
    """[L, 144, D] -> gappy [128, L*32] fp32 from children rows 0:80."""
    L = w_slice.shape[0]
    ch = w_slice[:, :80, :].reshape(L, 4, 20, D)
    out = np.zeros((L, 4, 32, 32), np.float32)
    out[:, :, :20, :D] = ch
    out = out.reshape(L, 128, 32).transpose(1, 0, 2)
    return np.ascontiguousarray(out).reshape(128, L * 32).astype(_bf16)


def _gappy_cols(vec_slice):
    """[L, D] -> [128, L/4] with row 32j+d, col q = vec[4q+j, d]; gaps zero."""
    L = vec_slice.shape[0]
    arr = vec_slice.reshape(L // 4, 4, D)
    out = np.zeros((L // 4, 4, 32), np.float32)
    out[:, :, :D] = arr
    out = out.reshape(L // 4, 128).T
    return np.ascontiguousarray(out)


def _prep_core(c, iv):
    s3 = slice(L3 * c, L3 * (c + 1))
    s2 = slice(L2 * c, L2 * (c + 1))
    s1 = slice(L1 * c, L1 * (c + 1))

    w0 = iv['W0'][0]                                    # [2624, 20]
    w0h = w0[:T1 * D, :].reshape(T1, D, D)[L1 * c:L1 * (c + 1)]   # [16, 20, 20]
    arr = w0h.reshape(Q1, 4, 20, D)
    w0c = np.zeros((Q1, 4, 32, D), np.float32)
    w0c[:, :, :20, :] = arr
    w0c = w0c.reshape(Q1, 128, D).transpose(1, 0, 2)
    w0c = np.ascontiguousarray(w0c).reshape(128, Q1 * D).astype(_bf16)

    hw0hb = np.zeros((33, 1), np.float32)
    hw0hb[:20, 0] = iv['hw0'][0][:, 0]
    hw0hb[32, 0] = iv['hb0'].reshape(-1)[0]

    return {
        'gt3': _genes_pairs(iv['genes3'][:, s3, :], 2 * CHUNK),
        'gt2': _genes_pairs(iv['genes2'][:, s2, :], 2 * CHUNK),
        'gt1': _genes_pairs(iv['genes1'][:, s1, :], P1)[0],
        'gt0': np.ascontiguousarray(iv['genes0'][:, 0, :].T).astype(_bf16),
        'w3': _w_pairs(iv['W3'][s3]),
        'w2c': _w_children(iv['W2'][s2]),
        'w2g': _w_pairs(iv['W2'][s2][:, 80:144, :]),
        'w1c': _w_children(iv['W1'][s1]),
        'w1g': _w_pairs(iv['W1'][s1][:, 80:144, :]),
        'w0c': w0c,
        'w0g': (w0[T1 * D:, :] / NCORES).astype(_bf16),
        'g3b': _gappy_cols(iv['g3'][s3]), 'be3b': _gappy_cols(iv['be3'][s3]),
        'g2b': _gappy_cols(iv['g2'][s2]), 'be2b': _gappy_cols(iv['be2'][s2]),
        'g1b': _gappy_cols(iv['g1'][s1]), 'be1b': _gappy_cols(iv['be1'][s1]),
        'g0c': np.ascontiguousarray(iv['g0'].reshape(1, D).T),
        'be0c': np.ascontiguousarray(iv['be0'].reshape(1, D).T),
        'hw0hb': hw0hb,
    }


def _prep_inputs(inputs):
    iv = {k: np.asarray(v, dtype=np.float32) for k, v in inputs.items()}
    return [_prep_core(c, iv) for c in range(NCORES)]


def run(in_maps, **kwargs):
    nc = _program()
    return run_bass_kernel_spmd(nc, in_maps, core_ids=list(range(NCORES)), **kwargs)


def kernel(**inputs) -> np.ndarray:
    in_maps = _prep_inputs(inputs)
    res = run(in_maps)
    pred = np.asarray(res.results[0]['out'], dtype=np.float32)   # [1, B]
    return np.ascontiguousarray(pred.T)                          # [B, 1]


# revision 17
# speedup vs baseline: 1.2305x; 1.1576x over previous
"""DCell hierarchy kernel for 8 Trainium2 NeuronCores.

Term-parallel: each core owns 1/8 of strata 3/2/1 (256/64/16 terms).
Activations live on-chip in quad tiles [128, B=256] (term j of the quad at
partitions 32j..32j+20, batch on the free axis).

Key points vs the original baseline:
- Correctness gate is 2e-2; the all-bf16 network measures ~6e-3 in fp64
  sim, so no hi/lo weight splitting anywhere.  Gene matmuls are 2-term
  block-diagonal pairs: stationary [128, 64] holds term A's weights on
  K-rows 0-63 and term B's on 64-127; the moving gene tile [128, B] stacks
  the two terms' gene states.  Halves both gene DMA and PE rows.
- BN: bn_aggr is gone -- mean/var come straight from bn_stats' 6-stat
  layout (count/mean/M2 for even and odd elements), with chunk-batched ALU
  ops on GPSIMD (int-typed rsqrt seed ops on DVE, which Pool can't codegen).
- Software pipelining: each chunk's smalls+tanh-applies are emitted one
  chunk behind its matmuls+stats, so DVE never stalls on the GPSIMD
  round-trip and the PE stays dense.
- Weights arrive as one consolidated blob DMA (fp32 pieces bitcast to bf16
  pairs) + per-chunk w3/gene tiles, cutting ~15 serial DGE dispatches.
- A dummy 64B AllReduce fires at kernel start so the CC firmware's
  rendezvous cost overlaps compute instead of sitting on the final
  AllReduce's critical path.
- Root head folds hb0 as an extra K-row (ones row at partition 32 of h0).
"""
import sys
sys.path.insert(0, '/opt/trn_rl_repo')

import numpy as np
import ml_dtypes

import concourse.bass as bass
import concourse.bacc as bacc
import concourse.mybir as mybir
from concourse import tile
from concourse.bass_utils import run_bass_kernel_spmd

F32 = mybir.dt.float32
BF16 = mybir.dt.bfloat16
AF = mybir.ActivationFunctionType
ALU = mybir.AluOpType

B, G, D = 256, 64, 20
T3, T2, T1 = 2048, 512, 128
FAN, EPS, NCORES = 4, 1e-5, 8
L3, L2, L1 = T3 // NCORES, T2 // NCORES, T1 // NCORES   # 256, 64, 16
Q3, Q2, Q1 = L3 // 4, L2 // 4, L1 // 4                  # 64, 16, 4
P3, P2, P1 = L3 // 2, L2 // 2, L1 // 2                  # 128, 32, 8 pairs
CHUNK = 8                                               # quads per BN chunk
NEWTON = 1                                              # rsqrt Newton steps

_bf16 = ml_dtypes.bfloat16

# blob column offsets (bf16 units; fp32 pieces use 2 cols per element)
_BL = {}
_off = 0
for _name, _cols in (("w2c", L2 * 32), ("w2g", P2 * 64), ("w1c", L1 * 32),
                     ("w1g", P1 * 64), ("w0c", Q1 * 20), ("gt1", P1 * B),
                     ("gt0", B), ("w0g", 20), ("pad0", 4),
                     ("g2b", 2 * Q2), ("be2b", 2 * Q2),
                     ("g1b", 2 * Q1), ("be1b", 2 * Q1),
                     ("g0c", 2), ("be0c", 2), ("hw0hb", 2)):
    _BL[_name] = (_off, _off + _cols)
    _off += _cols
BLOB_COLS = _off


# --------------------------------------------------------------------------
# device program
# --------------------------------------------------------------------------

def _build_program():
    nc = bacc.Bacc(None, target_bir_lowering=False, debug=False)

    gt3_d = nc.dram_tensor("gt3", [Q3 // CHUNK, 128, 2 * CHUNK * B], BF16,
                           kind="ExternalInput")
    w3_d = nc.dram_tensor("w3", [Q3 // CHUNK, 128, 2 * CHUNK * 64], BF16,
                          kind="ExternalInput")
    gt2_d = nc.dram_tensor("gt2", [Q2 // CHUNK, 128, 2 * CHUNK * B], BF16,
                           kind="ExternalInput")
    g3_d = nc.dram_tensor("g3b", [128, Q3], F32, kind="ExternalInput")
    be3_d = nc.dram_tensor("be3b", [128, Q3], F32, kind="ExternalInput")
    blob_d = nc.dram_tensor("blob", [128, BLOB_COLS], BF16,
                            kind="ExternalInput")
    out_d = nc.dram_tensor("out", [1, B], F32, kind="ExternalOutput")

    with tile.TileContext(nc) as tc:
        with tc.tile_pool(name="const", bufs=1) as cp, \
             tc.tile_pool(name="gin", bufs=3) as gp, \
             tc.tile_pool(name="hbuf", bufs=1) as hp, \
             tc.tile_pool(name="stat", bufs=1) as sp, \
             tc.tile_pool(name="zps", bufs=8, space="PSUM") as zp, \
             tc.tile_pool(name="dram", bufs=1, space="DRAM") as dp:

            # dummy collective to warm the CC firmware, overlapped with
            # compute (no dependency on anything)
            ccw_in = dp.tile([1, 16], F32)
            ccw_out = dp.tile([1, 16], F32, addr_space="Shared")
            warm = sp.tile([1, 16], F32)
            nc.vector.memset(warm[:], 0.0)
            nc.gpsimd.dma_start(out=ccw_in[:], in_=warm[:])
            nc.gpsimd.collective_compute(
                "AllReduce", ALU.add,
                replica_groups=[list(range(NCORES))],
                ins=[ccw_in.opt()], outs=[ccw_out.opt()])

            gamma3 = cp.tile([128, Q3], F32)
            nc.sync.dma_start(out=gamma3[:], in_=g3_d[:])
            beta3 = cp.tile([128, Q3], F32)
            nc.sync.dma_start(out=beta3[:], in_=be3_d[:])

            # ---- activation + stat buffers ----
            h3b = hp.tile([128, Q3 * B], BF16)
            h2b = hp.tile([128, Q2 * B], BF16)
            h1b = hp.tile([128, Q1 * B], BF16)
            hbuf = {3: h3b, 2: h2b, 1: h1b}
            stats = {}
            for s, q in ((3, Q3), (2, Q2), (1, Q1)):
                stats[s] = dict(
                    st=sp.tile([128, 6 * q], F32, name=f"st{s}"),
                    ssum=sp.tile([128, q], F32, name=f"ssum{s}"),
                    sdif=sp.tile([128, q], F32, name=f"sdif{s}"),
                    vex=sp.tile([128, q], F32, name=f"vex{s}"),
                    inv=sp.tile([128, q], F32, name=f"inv{s}"),
                    tm=sp.tile([128, q], F32, name=f"tm{s}"),
                    nt=sp.tile([128, q], F32, name=f"nt{s}"),
                    sc=sp.tile([128, q], F32, name=f"sc{s}"),
                    bi=sp.tile([128, q], F32, name=f"bi{s}"),
                )

            gb = {}
            eng = nc.gpsimd

            def bn_smalls(s, q0, n):
                """Scale/bias for quads q0..q0+n of stratum s, straight from
                the 6-stat (count,mean,M2)x{even,odd} layout:
                mean = (me+mo)/2,  var = (M2e+M2o)/256 + ((me-mo)/2)^2."""
                S = stats[s]
                gam, bet = gb[s]
                st = S['st']
                me = st[:, 6 * q0 + 1: 6 * (q0 + n): 6]
                mo = st[:, 6 * q0 + 4: 6 * (q0 + n): 6]
                cve = st[:, 6 * q0 + 2: 6 * (q0 + n): 6]
                cvo = st[:, 6 * q0 + 5: 6 * (q0 + n): 6]
                sl = slice(q0, q0 + n)
                ssum, sdif = S['ssum'][:, sl], S['sdif'][:, sl]
                vex, inv = S['vex'][:, sl], S['inv'][:, sl]
                tm, nt = S['tm'][:, sl], S['nt'][:, sl]
                sc, bi = S['sc'][:, sl], S['bi'][:, sl]
                eng.tensor_tensor(ssum, me, mo, op=ALU.add)
                eng.tensor_tensor(sdif, me, mo, op=ALU.subtract)
                eng.tensor_tensor(vex, cve, cvo, op=ALU.add)
                eng.tensor_tensor(tm, sdif, sdif, op=ALU.mult)
                eng.tensor_scalar(vex, vex, 1.0 / B, None, op0=ALU.mult)
                eng.tensor_scalar(tm, tm, 0.25, EPS, op0=ALU.mult, op1=ALU.add)
                eng.tensor_tensor(vex, vex, tm, op=ALU.add)
                # rsqrt: magic-constant seed (int ops, DVE-only) + Newton
                iv = inv.bitcast(mybir.dt.int32)
                nc.vector.tensor_scalar(iv, vex.bitcast(mybir.dt.int32), 1, -1,
                                        op0=ALU.arith_shift_right,
                                        op1=ALU.bitwise_xor)
                nc.vector.tensor_scalar(iv, iv, 0x5f3759e0, None, op0=ALU.add)
                for _ in range(NEWTON):
                    eng.tensor_tensor(nt, inv, inv, op=ALU.mult)
                    eng.tensor_tensor(nt, nt, vex, op=ALU.mult)
                    eng.tensor_scalar(nt, nt, -0.5, 1.5, op0=ALU.mult,
                                      op1=ALU.add)
                    eng.tensor_tensor(inv, inv, nt, op=ALU.mult)
                eng.tensor_tensor(sc, inv, gam[:, sl], op=ALU.mult)
                eng.tensor_tensor(tm, ssum, sc, op=ALU.mult)
                eng.tensor_scalar(tm, tm, 0.5, None, op0=ALU.mult)
                eng.tensor_tensor(bi, bet[:, sl], tm, op=ALU.subtract)

            def bn_apply(s, zq, q):
                S = stats[s]
                nc.scalar.activation(hbuf[s][:, B * q:B * (q + 1)], zq, AF.Tanh,
                                     bias=S['bi'][:, q:q + 1],
                                     scale=S['sc'][:, q:q + 1])

            def flush(item):
                if item is None:
                    return
                s, q0, n, pend = item
                bn_smalls(s, q0, n)
                for zq, q in pend:
                    bn_apply(s, zq, q)

            pending = None
            blob = None

            # ================= stratum 3 =================
            for c in range(Q3 // CHUNK):
                w3t = gp.tile([128, 2 * CHUNK * 64], BF16, name="w3t",
                              tag="w3t")
                nc.sync.dma_start(out=w3t[:], in_=w3_d[c, :, :])
                gt3t = gp.tile([128, 2 * CHUNK * B], BF16, name="gt3t",
                               tag="gt3t")
                nc.sync.dma_start(out=gt3t[:], in_=gt3_d[c, :, :])
                pend = []
                for qq in range(CHUNK):
                    q = c * CHUNK + qq
                    if qq % 2 == 0:
                        zpair = zp.tile([128, 2, B], F32, name="z3t", tag="z")
                    zq = zpair[:, qq % 2, :]
                    for half in range(2):
                        p = 2 * q + half            # pair index
                        slot = p - 2 * c * CHUNK    # slot in this chunk tile
                        nc.tensor.matmul(zq[64 * half:64 * half + 64, :],
                                         w3t[:, 64 * slot:64 * (slot + 1)],
                                         gt3t[:, B * slot:B * (slot + 1)],
                                         start=True, stop=True,
                                         tile_position=(0, 64 * half))
                    pend.append((zq, q))
                    nc.vector.bn_stats(stats[3]['st'][:, 6 * q:6 * q + 6], zq)
                flush(pending)
                pending = (3, c * CHUNK, CHUNK, pend)

                if c == 0:
                    gb[3] = (gamma3, beta3)
                    blob = cp.tile([128, BLOB_COLS], BF16)
                    nc.sync.dma_start(out=blob[:], in_=blob_d[:])

                    def bl(name, dt=BF16):
                        a, b = _BL[name]
                        v = blob[:, a:b]
                        return v.bitcast(F32) if dt == F32 else v

                    w2c, w2g = bl("w2c"), bl("w2g")
                    w1c, w1g = bl("w1c"), bl("w1g")
                    w0c, w0g = bl("w0c"), bl("w0g")
                    gt1, gt0 = bl("gt1"), bl("gt0")
                    gb[2] = (bl("g2b", F32), bl("be2b", F32))
                    gb[1] = (bl("g1b", F32), bl("be1b", F32))
                    g0c = bl("g0c", F32)[0:20, :]
                    be0c = bl("be0c", F32)[0:20, :]
                    hw0hb = bl("hw0hb")[0:33, 0:1]

            g2tiles = []
            for grp in range(Q2 // CHUNK):
                g2t = gp.tile([128, 2 * CHUNK * B], BF16, name="gt2t",
                              tag="gt2t", bufs=2)
                nc.sync.dma_start(out=g2t[:], in_=gt2_d[grp, :, :])
                g2tiles.append(g2t)

            # ================= strata 2 and 1 =================
            def mid_stratum(s, nq, wc, wg, gtile_lookup, flush_first=False):
                nonlocal pending
                prev = hbuf[s + 1]
                if flush_first:
                    # this stratum's first chunk reads activations whose
                    # applies are still pending; program order must put the
                    # writes first
                    flush(pending)
                    pending = None
                for c0 in range(0, nq, CHUNK):
                    nch = min(CHUNK, nq - c0)
                    pend = []
                    for qq in range(nch):
                        q = c0 + qq
                        if qq % 2 == 0:
                            zpair = zp.tile([128, 2, B], F32, name=f"z{s}t",
                                            tag="z")
                        zq = zpair[:, qq % 2, :]
                        # gene pair matmuls open the bank (their zero weight
                        # rows also zero the gap partitions), children
                        # accumulate on top.
                        for half in range(2):
                            p = 2 * q + half
                            gt_, slot = gtile_lookup(p)
                            nc.tensor.matmul(zq[64 * half:64 * half + 64, :],
                                             wg[:, 64 * p:64 * p + 64],
                                             gt_[:, B * slot:B * (slot + 1)],
                                             start=True, stop=False,
                                             tile_position=(0, 64 * half),
                                             skip_group_check=True)
                        for j in range(4):
                            u = 4 * q + j
                            nc.tensor.matmul(
                                zq[32 * j:32 * j + 32, :],
                                wc[:, 32 * u:32 * u + 32],
                                prev[:, B * u:B * (u + 1)],
                                start=False, stop=True,
                                tile_position=(0, 32 * j),
                                skip_group_check=True)
                        pend.append((zq, q))
                        nc.vector.bn_stats(stats[s]['st'][:, 6 * q:6 * q + 6],
                                           zq)
                    flush(pending)
                    pending = (s, c0, nch, pend)

            mid_stratum(2, Q2, w2c, w2g,
                        lambda p: (g2tiles[p // (2 * CHUNK)],
                                   p % (2 * CHUNK)))
            mid_stratum(1, Q1, w1c, w1g, lambda p: (gt1, p),
                        flush_first=True)
            flush(pending)
            pending = None

            # ================= root =================
            zr = zp.tile([20, B], F32, name="zr", tag="z")
            for q1 in range(Q1):
                nc.tensor.matmul(zr[:], w0c[:, 20 * q1:20 * (q1 + 1)],
                                 h1b[:, B * q1:B * (q1 + 1)],
                                 start=(q1 == 0), stop=False)
            nc.tensor.matmul(zr[:], w0g[0:64, :], gt0[0:64, :],
                             start=False, stop=True)

            z0p = sp.tile([20, B], F32)
            nc.vector.tensor_copy(z0p[:], zr[:])

            cc_in = dp.tile([20, B], F32)
            cc_out = dp.tile([20, B], F32, addr_space="Shared")
            nc.gpsimd.dma_start(out=cc_in[:], in_=z0p[:])
            nc.gpsimd.collective_compute(
                "AllReduce", ALU.add,
                replica_groups=[list(range(NCORES))],
                ins=[cc_in.opt()], outs=[cc_out.opt()])
            z0 = sp.tile([20, B], F32)
            nc.sync.dma_start(out=z0[:], in_=cc_out[:])

            # root BN: single bn_stats, mean/var from the 6-stat layout
            st0 = sp.tile([20, 6], F32)
            nc.vector.bn_stats(st0[:], z0[:])
            me0, mo0 = st0[:, 1:2], st0[:, 4:5]
            cve0, cvo0 = st0[:, 2:3], st0[:, 5:6]
            s0 = sp.tile([20, 1], F32)
            d0 = sp.tile([20, 1], F32)
            v0 = sp.tile([20, 1], F32)
            i0 = sp.tile([20, 1], F32)
            t0 = sp.tile([20, 1], F32)
            n0 = sp.tile([20, 1], F32)
            V = nc.vector
            V.tensor_tensor(s0[:], me0, mo0, op=ALU.add)
            V.tensor_tensor(d0[:], me0, mo0, op=ALU.subtract)
            V.tensor_tensor(v0[:], cve0, cvo0, op=ALU.add)
            V.tensor_tensor(t0[:], d0[:], d0[:], op=ALU.mult)
            V.tensor_scalar(v0[:], v0[:], 1.0 / B, None, op0=ALU.mult)
            V.tensor_scalar(t0[:], t0[:], 0.25, EPS, op0=ALU.mult, op1=ALU.add)
            V.tensor_tensor(v0[:], v0[:], t0[:], op=ALU.add)
            iv0 = i0[:].bitcast(mybir.dt.int32)
            V.tensor_scalar(iv0, v0[:].bitcast(mybir.dt.int32), 1, -1,
                            op0=ALU.arith_shift_right, op1=ALU.bitwise_xor)
            V.tensor_scalar(iv0, iv0, 0x5f3759e0, None, op0=ALU.add)
            for _ in range(2):
                V.tensor_tensor(n0[:], i0[:], i0[:], op=ALU.mult)
                V.tensor_tensor(n0[:], n0[:], v0[:], op=ALU.mult)
                V.tensor_scalar(n0[:], n0[:], -0.5, 1.5, op0=ALU.mult,
                                op1=ALU.add)
                V.tensor_tensor(i0[:], i0[:], n0[:], op=ALU.mult)
            sc0 = sp.tile([20, 1], F32)
            V.tensor_tensor(sc0[:], i0[:], g0c[:], op=ALU.mult)
            V.tensor_tensor(t0[:], s0[:], sc0[:], op=ALU.mult)
            V.tensor_scalar(t0[:], t0[:], 0.5, None, op0=ALU.mult)
            bi0 = sp.tile([20, 1], F32)
            V.tensor_tensor(bi0[:], be0c[:], t0[:], op=ALU.subtract)

            # h0 with a ones row at partition 32 so the bf16 head matmul
            # folds hb0 (rows 20..31 zeroed once).
            h0 = sp.tile([33, B], BF16)
            nc.vector.memset(h0[0:33, :], 0.0)
            nc.vector.memset(h0[32:33, :], 1.0)
            nc.scalar.activation(h0[0:20, :], z0[:], AF.Tanh,
                                 bias=bi0[:], scale=sc0[:])
            zh = zp.tile([1, B], F32, name="zh", tag="z")
            nc.tensor.matmul(zh[:], hw0hb[:], h0[:], start=True, stop=True)
            osb = sp.tile([1, B], F32)
            nc.vector.tensor_copy(osb[:], zh[:])
            nc.sync.dma_start(out=out_d[:], in_=osb[:])

    nc.compile()
    return nc


_PROGRAM = None


def _program():
    global _PROGRAM
    if _PROGRAM is None:
        _PROGRAM = _build_program()
    return _PROGRAM


# --------------------------------------------------------------------------
# host-side sharding / layout
# --------------------------------------------------------------------------

def _genes_pairs(genes_slice, group):
    """[B, T, G] fp32 -> pair tiles: [T//(2*group), 128, group*B] bf16.

    Pair p stacks term 2p's genes on K-rows 0-63 and term 2p+1's on 64-127.
    `group` pairs are packed per DMA tile."""
    t = genes_slice.shape[1]
    x = np.ascontiguousarray(genes_slice.transpose(1, 2, 0))      # [T, G, B]
    x = x.reshape(t // 2, 128, B)                                  # pairs
    p = t // 2
    x = x.reshape(p // group, group, 128, B).transpose(0, 2, 1, 3)
    return np.ascontiguousarray(x).reshape(p // group, 128, group * B) \
        .astype(_bf16)


def _w_pairs(w_slice):
    """[L, 64, D] gene weights -> [128, (L/2)*64] bf16 block-diag pairs."""
    L = w_slice.shape[0]
    out = np.zeros((L // 2, 128, 64), np.float32)
    out[:, 0:64, 0:D] = w_slice[0::2]
    out[:, 64:128, 32:32 + D] = w_slice[1::2]
    out = out.transpose(1, 0, 2)
    return np.ascontiguousarray(out).reshape(128, (L // 2) * 64).astype(_bf16)


def _w_children(w_slice):
    """[L, 144, D] -> gappy [128, L*32] bf16 from children rows 0:80."""
    L = w_slice.shape[0]
    ch = w_slice[:, :80, :].reshape(L, 4, 20, D)
    out = np.zeros((L, 4, 32, 32), np.float32)
    out[:, :, :20, :D] = ch
    out = out.reshape(L, 128, 32).transpose(1, 0, 2)
    return np.ascontiguousarray(out).reshape(128, L * 32).astype(_bf16)


def _gappy_cols(vec_slice):
    """[L, D] -> [128, L/4] f32 with row 32j+d, col q = vec[4q+j, d]."""
    L = vec_slice.shape[0]
    arr = vec_slice.reshape(L // 4, 4, D)
    out = np.zeros((L // 4, 4, 32), np.float32)
    out[:, :, :D] = arr
    out = out.reshape(L // 4, 128).T
    return np.ascontiguousarray(out)


def _f32_to_bf2(a):
    """fp32 array -> byte-identical bf16 view with doubled last dim."""
    return np.ascontiguousarray(a.astype(np.float32)).view(_bf16)


def _prep_core(c, iv):
    s3 = slice(L3 * c, L3 * (c + 1))
    s2 = slice(L2 * c, L2 * (c + 1))
    s1 = slice(L1 * c, L1 * (c + 1))

    w0 = iv['W0'][0]                                    # [2624, 20]
    w0h = w0[:T1 * D, :].reshape(T1, D, D)[L1 * c:L1 * (c + 1)]   # [16, 20, 20]
    arr = w0h.reshape(Q1, 4, 20, D)
    w0c = np.zeros((Q1, 4, 32, D), np.float32)
    w0c[:, :, :20, :] = arr
    w0c = w0c.reshape(Q1, 128, D).transpose(1, 0, 2)
    w0c = np.ascontiguousarray(w0c).reshape(128, Q1 * D).astype(_bf16)

    hw0hb = np.zeros((33, 1), np.float32)
    hw0hb[:20, 0] = iv['hw0'][0][:, 0]
    hw0hb[32, 0] = iv['hb0'].reshape(-1)[0]

    w3p = _w_pairs(iv['W3'][s3])                        # [128, P3*64]
    w3ch = w3p.reshape(128, Q3 // CHUNK, 2 * CHUNK * 64).transpose(1, 0, 2)
    w3ch = np.ascontiguousarray(w3ch)

    gt0 = np.zeros((128, B), _bf16)
    gt0[0:64, :] = iv['genes0'][:, 0, :].T.astype(_bf16)
    w0g = np.zeros((128, 20), _bf16)
    w0g[0:64, :] = (w0[T1 * D:, :] / NCORES).astype(_bf16)

    def pad128(a20, rows):
        out = np.zeros((128, a20.shape[1]), np.float32)
        out[0:rows] = a20
        return out

    blob = np.zeros((128, BLOB_COLS), _bf16)

    def put(name, arr):
        a, b = _BL[name]
        assert arr.shape[1] == b - a, (name, arr.shape, b - a)
        blob[:, a:b] = arr

    put("w2c", _w_children(iv['W2'][s2]))
    put("w2g", _w_pairs(iv['W2'][s2][:, 80:144, :]))
    put("w1c", _w_children(iv['W1'][s1]))
    put("w1g", _w_pairs(iv['W1'][s1][:, 80:144, :]))
    put("w0c", w0c)
    put("gt1", _genes_pairs(iv['genes1'][:, s1, :], P1)[0])
    put("gt0", gt0)
    put("w0g", w0g)
    put("g2b", _f32_to_bf2(_gappy_cols(iv['g2'][s2])))
    put("be2b", _f32_to_bf2(_gappy_cols(iv['be2'][s2])))
    put("g1b", _f32_to_bf2(_gappy_cols(iv['g1'][s1])))
    put("be1b", _f32_to_bf2(_gappy_cols(iv['be1'][s1])))
    put("g0c", _f32_to_bf2(pad128(iv['g0'].reshape(1, D).T, 20)))
    put("be0c", _f32_to_bf2(pad128(iv['be0'].reshape(1, D).T, 20)))
    hwb = np.zeros((128, 2), _bf16)
    hwb[0:33, 0:1] = pad128(hw0hb, 33)[0:33].astype(_bf16)
    put("hw0hb", hwb)

    return {
        'gt3': _genes_pairs(iv['genes3'][:, s3, :], 2 * CHUNK),
        'gt2': _genes_pairs(iv['genes2'][:, s2, :], 2 * CHUNK),
        'w3': w3ch,
        'g3b': _gappy_cols(iv['g3'][s3]),
        'be3b': _gappy_cols(iv['be3'][s3]),
        'blob': blob,
    }


def _prep_inputs(inputs):
    iv = {k: np.asarray(v, dtype=np.float32) for k, v in inputs.items()}
    return [_prep_core(c, iv) for c in range(NCORES)]


def run(in_maps, **kwargs):
    nc = _program()
    return run_bass_kernel_spmd(nc, in_maps, core_ids=list(range(NCORES)), **kwargs)


def kernel(**inputs) -> np.ndarray:
    in_maps = _prep_inputs(inputs)
    res = run(in_maps)
    pred = np.asarray(res.results[0]['out'], dtype=np.float32)   # [1, B]
    return np.ascontiguousarray(pred.T)                          # [B, 1]


# revision 18
# speedup vs baseline: 1.3240x; 1.0759x over previous
"""DCell hierarchy kernel for 8 Trainium2 NeuronCores.

Term-parallel: each core owns 1/8 of strata 3/2/1 (256/64/16 terms).
Activations live on-chip in quad tiles [128, B=256] (term j of the quad at
partitions 32j..32j+20, batch on the free axis).

Key points vs the original baseline:
- Correctness gate is 2e-2; the all-bf16 network measures ~6e-3 in fp64
  sim, so no hi/lo weight splitting anywhere.  Gene matmuls are 2-term
  block-diagonal pairs: stationary [128, 64] holds term A's weights on
  K-rows 0-63 and term B's on 64-127; the moving gene tile [128, B] stacks
  the two terms' gene states.  Halves both gene DMA and PE rows.
- BN: bn_aggr is gone -- mean/var come straight from bn_stats' 6-stat
  layout (count/mean/M2 for even and odd elements), with chunk-batched ALU
  ops on GPSIMD (int-typed rsqrt seed ops on DVE, which Pool can't codegen).
- Software pipelining: each chunk's smalls+tanh-applies are emitted one
  chunk behind its matmuls+stats, so DVE never stalls on the GPSIMD
  round-trip and the PE stays dense.
- Weights arrive as one consolidated blob DMA (fp32 pieces bitcast to bf16
  pairs) + per-chunk w3/gene tiles, cutting ~15 serial DGE dispatches.
- A dummy 64B AllReduce fires at kernel start so the CC firmware's
  rendezvous cost overlaps compute instead of sitting on the final
  AllReduce's critical path.
- Root head folds hb0 as an extra K-row (ones row at partition 32 of h0).
"""
import sys
sys.path.insert(0, '/opt/trn_rl_repo')

import numpy as np
import ml_dtypes

import concourse.bass as bass
import concourse.bacc as bacc
import concourse.mybir as mybir
from concourse import tile
from concourse.bass_utils import run_bass_kernel_spmd

F32 = mybir.dt.float32
BF16 = mybir.dt.bfloat16
F8 = mybir.dt.float8e4
AF = mybir.ActivationFunctionType
ALU = mybir.AluOpType

B, G, D = 256, 64, 20
T3, T2, T1 = 2048, 512, 128
FAN, EPS, NCORES = 4, 1e-5, 8
L3, L2, L1 = T3 // NCORES, T2 // NCORES, T1 // NCORES   # 256, 64, 16
Q3, Q2, Q1 = L3 // 4, L2 // 4, L1 // 4                  # 64, 16, 4
P3, P2, P1 = L3 // 2, L2 // 2, L1 // 2                  # 128, 32, 8 pairs
CHUNK = 8                                               # quads per BN chunk
NEWTON = 1                                              # rsqrt Newton steps

_bf16 = ml_dtypes.bfloat16
_f8 = ml_dtypes.float8_e4m3

# blob column offsets (bf16 units; fp32 pieces use 2 cols per element)
_BL = {}
_off = 0
for _name, _cols in (("w2c", L2 * 32), ("w2g", P2 * 64), ("w1c", L1 * 32),
                     ("w1g", P1 * 64), ("w0c", Q1 * 20), ("gt1", P1 * B // 2),
                     ("gt0", B // 2), ("w0g", 20), ("pad0", 4),
                     ("g2b", 2 * Q2), ("be2b", 2 * Q2),
                     ("g1b", 2 * Q1), ("be1b", 2 * Q1),
                     ("g0c", 2), ("be0c", 2), ("hw0hb", 2)):
    _BL[_name] = (_off, _off + _cols)
    _off += _cols
BLOB_COLS = _off


# --------------------------------------------------------------------------
# device program
# --------------------------------------------------------------------------

def _build_program():
    nc = bacc.Bacc(None, target_bir_lowering=False, debug=False)

    gt3_d = nc.dram_tensor("gt3", [Q3 // CHUNK, 128, 2 * CHUNK * B], F8,
                           kind="ExternalInput")
    w3_d = nc.dram_tensor("w3", [Q3 // CHUNK, 128, 2 * CHUNK * 64], BF16,
                          kind="ExternalInput")
    gt2_d = nc.dram_tensor("gt2", [Q2 // CHUNK, 128, 2 * CHUNK * B], F8,
                           kind="ExternalInput")
    g3_d = nc.dram_tensor("g3b", [128, Q3], F32, kind="ExternalInput")
    be3_d = nc.dram_tensor("be3b", [128, Q3], F32, kind="ExternalInput")
    blob_d = nc.dram_tensor("blob", [128, BLOB_COLS], BF16,
                            kind="ExternalInput")
    out_d = nc.dram_tensor("out", [1, B], F32, kind="ExternalOutput")

    with tile.TileContext(nc) as tc:
        with tc.tile_pool(name="const", bufs=1) as cp, \
             tc.tile_pool(name="gin", bufs=3) as gp, \
             tc.tile_pool(name="hbuf", bufs=1) as hp, \
             tc.tile_pool(name="stat", bufs=1) as sp, \
             tc.tile_pool(name="zps", bufs=8, space="PSUM") as zp, \
             tc.tile_pool(name="dram", bufs=1, space="DRAM") as dp:

            # dummy collective to warm the CC firmware, overlapped with
            # compute (no dependency on anything)
            ccw_in = dp.tile([1, 16], F32)
            ccw_out = dp.tile([1, 16], F32, addr_space="Shared")
            warm = sp.tile([1, 16], F32)
            nc.vector.memset(warm[:], 0.0)
            nc.gpsimd.dma_start(out=ccw_in[:], in_=warm[:])
            nc.gpsimd.collective_compute(
                "AllReduce", ALU.add,
                replica_groups=[list(range(NCORES))],
                ins=[ccw_in.opt()], outs=[ccw_out.opt()])

            gamma3 = cp.tile([128, Q3], F32)
            nc.sync.dma_start(out=gamma3[:], in_=g3_d[:])
            beta3 = cp.tile([128, Q3], F32)
            nc.sync.dma_start(out=beta3[:], in_=be3_d[:])

            # ---- activation + stat buffers ----
            h3b = hp.tile([128, Q3 * B], BF16)
            h2b = hp.tile([128, Q2 * B], BF16)
            h1b = hp.tile([128, Q1 * B], BF16)
            hbuf = {3: h3b, 2: h2b, 1: h1b}
            stats = {}
            for s, q in ((3, Q3), (2, Q2), (1, Q1)):
                stats[s] = dict(
                    st=sp.tile([128, 6 * q], F32, name=f"st{s}"),
                    ssum=sp.tile([128, q], F32, name=f"ssum{s}"),
                    sdif=sp.tile([128, q], F32, name=f"sdif{s}"),
                    vex=sp.tile([128, q], F32, name=f"vex{s}"),
                    inv=sp.tile([128, q], F32, name=f"inv{s}"),
                    tm=sp.tile([128, q], F32, name=f"tm{s}"),
                    nt=sp.tile([128, q], F32, name=f"nt{s}"),
                    sc=sp.tile([128, q], F32, name=f"sc{s}"),
                    bi=sp.tile([128, q], F32, name=f"bi{s}"),
                )

            gb = {}
            eng = nc.gpsimd

            def bn_smalls(s, q0, n):
                """Scale/bias for quads q0..q0+n of stratum s, straight from
                the 6-stat (count,mean,M2)x{even,odd} layout:
                mean = (me+mo)/2,  var = (M2e+M2o)/256 + ((me-mo)/2)^2."""
                S = stats[s]
                gam, bet = gb[s]
                st = S['st']
                me = st[:, 6 * q0 + 1: 6 * (q0 + n): 6]
                mo = st[:, 6 * q0 + 4: 6 * (q0 + n): 6]
                cve = st[:, 6 * q0 + 2: 6 * (q0 + n): 6]
                cvo = st[:, 6 * q0 + 5: 6 * (q0 + n): 6]
                sl = slice(q0, q0 + n)
                ssum, sdif = S['ssum'][:, sl], S['sdif'][:, sl]
                vex, inv = S['vex'][:, sl], S['inv'][:, sl]
                tm, nt = S['tm'][:, sl], S['nt'][:, sl]
                sc, bi = S['sc'][:, sl], S['bi'][:, sl]
                eng.tensor_tensor(ssum, me, mo, op=ALU.add)
                eng.tensor_tensor(sdif, me, mo, op=ALU.subtract)
                eng.tensor_tensor(vex, cve, cvo, op=ALU.add)
                eng.tensor_tensor(tm, sdif, sdif, op=ALU.mult)
                eng.tensor_scalar(vex, vex, 1.0 / B, None, op0=ALU.mult)
                eng.tensor_scalar(tm, tm, 0.25, EPS, op0=ALU.mult, op1=ALU.add)
                eng.tensor_tensor(vex, vex, tm, op=ALU.add)
                # rsqrt: magic-constant seed (int ops, DVE-only) + Newton
                iv = inv.bitcast(mybir.dt.int32)
                nc.vector.tensor_scalar(iv, vex.bitcast(mybir.dt.int32), 1, -1,
                                        op0=ALU.arith_shift_right,
                                        op1=ALU.bitwise_xor)
                nc.vector.tensor_scalar(iv, iv, 0x5f3759e0, None, op0=ALU.add)
                for _ in range(NEWTON):
                    eng.tensor_tensor(nt, inv, inv, op=ALU.mult)
                    eng.tensor_tensor(nt, nt, vex, op=ALU.mult)
                    eng.tensor_scalar(nt, nt, -0.5, 1.5, op0=ALU.mult,
                                      op1=ALU.add)
                    eng.tensor_tensor(inv, inv, nt, op=ALU.mult)
                eng.tensor_tensor(sc, inv, gam[:, sl], op=ALU.mult)
                eng.tensor_tensor(tm, ssum, sc, op=ALU.mult)
                eng.tensor_scalar(tm, tm, 0.5, None, op0=ALU.mult)
                eng.tensor_tensor(bi, bet[:, sl], tm, op=ALU.subtract)

            def bn_apply(s, zq, q):
                S = stats[s]
                nc.scalar.activation(hbuf[s][:, B * q:B * (q + 1)], zq, AF.Tanh,
                                     bias=S['bi'][:, q:q + 1],
                                     scale=S['sc'][:, q:q + 1])

            def flush(item):
                if item is None:
                    return
                s, q0, n, pend = item
                bn_smalls(s, q0, n)
                for zq, q in pend:
                    bn_apply(s, zq, q)

            pending = None
            blob = None

            # ================= stratum 3 =================
            for c in range(Q3 // CHUNK):
                w3t = gp.tile([128, 2 * CHUNK * 64], BF16, name="w3t",
                              tag="w3t")
                nc.sync.dma_start(out=w3t[:], in_=w3_d[c, :, :])
                gt3t = gp.tile([128, 2 * CHUNK * B], F8, name="gt3t",
                               tag="gt3t")
                nc.sync.dma_start(out=gt3t[:], in_=gt3_d[c, :, :])
                pend = []
                for qq in range(CHUNK):
                    q = c * CHUNK + qq
                    if qq % 2 == 0:
                        zpair = zp.tile([128, 2, B], F32, name="z3t", tag="z")
                    zq = zpair[:, qq % 2, :]
                    for half in range(2):
                        p = 2 * q + half            # pair index
                        slot = p - 2 * c * CHUNK    # slot in this chunk tile
                        nc.tensor.matmul(zq[64 * half:64 * half + 64, :],
                                         w3t[:, 64 * slot:64 * (slot + 1)],
                                         gt3t[:, B * slot:B * (slot + 1)],
                                         start=True, stop=True,
                                         tile_position=(0, 64 * half))
                    pend.append((zq, q))
                    nc.vector.bn_stats(stats[3]['st'][:, 6 * q:6 * q + 6], zq)
                flush(pending)
                pending = (3, c * CHUNK, CHUNK, pend)

                if c == 0:
                    gb[3] = (gamma3, beta3)
                    blob = cp.tile([128, BLOB_COLS], BF16)
                    nc.sync.dma_start(out=blob[:], in_=blob_d[:])

                    def bl(name, dt=BF16):
                        a, b = _BL[name]
                        v = blob[:, a:b]
                        return v.bitcast(F32) if dt == F32 else v

                    w2c, w2g = bl("w2c"), bl("w2g")
                    w1c, w1g = bl("w1c"), bl("w1g")
                    w0c, w0g = bl("w0c"), bl("w0g")
                    gt1 = bl("gt1").bitcast(F8)
                    gt0 = bl("gt0").bitcast(F8)
                    gb[2] = (bl("g2b", F32), bl("be2b", F32))
                    gb[1] = (bl("g1b", F32), bl("be1b", F32))
                    g0c = bl("g0c", F32)[0:20, :]
                    be0c = bl("be0c", F32)[0:20, :]
                    hw0hb = bl("hw0hb")[0:33, 0:1]

            g2tiles = []
            for grp in range(Q2 // CHUNK):
                g2t = gp.tile([128, 2 * CHUNK * B], F8, name="gt2t",
                              tag="gt2t", bufs=2)
                nc.sync.dma_start(out=g2t[:], in_=gt2_d[grp, :, :])
                g2tiles.append(g2t)

            # ================= strata 2 and 1 =================
            def mid_stratum(s, nq, wc, wg, gtile_lookup, flush_first=False):
                nonlocal pending
                prev = hbuf[s + 1]
                if flush_first:
                    # this stratum's first chunk reads activations whose
                    # applies are still pending; program order must put the
                    # writes first
                    flush(pending)
                    pending = None
                for c0 in range(0, nq, CHUNK):
                    nch = min(CHUNK, nq - c0)
                    pend = []
                    for qq in range(nch):
                        q = c0 + qq
                        if qq % 2 == 0:
                            zpair = zp.tile([128, 2, B], F32, name=f"z{s}t",
                                            tag="z")
                        zq = zpair[:, qq % 2, :]
                        # gene pair matmuls open the bank (their zero weight
                        # rows also zero the gap partitions), children
                        # accumulate on top.
                        for half in range(2):
                            p = 2 * q + half
                            gt_, slot = gtile_lookup(p)
                            nc.tensor.matmul(zq[64 * half:64 * half + 64, :],
                                             wg[:, 64 * p:64 * p + 64],
                                             gt_[:, B * slot:B * (slot + 1)],
                                             start=True, stop=False,
                                             tile_position=(0, 64 * half),
                                             skip_group_check=True)
                        for j in range(4):
                            u = 4 * q + j
                            nc.tensor.matmul(
                                zq[32 * j:32 * j + 32, :],
                                wc[:, 32 * u:32 * u + 32],
                                prev[:, B * u:B * (u + 1)],
                                start=False, stop=True,
                                tile_position=(0, 32 * j),
                                skip_group_check=True)
                        pend.append((zq, q))
                        nc.vector.bn_stats(stats[s]['st'][:, 6 * q:6 * q + 6],
                                           zq)
                    flush(pending)
                    pending = (s, c0, nch, pend)

            mid_stratum(2, Q2, w2c, w2g,
                        lambda p: (g2tiles[p // (2 * CHUNK)],
                                   p % (2 * CHUNK)))
            mid_stratum(1, Q1, w1c, w1g, lambda p: (gt1, p),
                        flush_first=True)
            flush(pending)
            pending = None

            # ================= root =================
            zr = zp.tile([20, B], F32, name="zr", tag="z")
            for q1 in range(Q1):
                nc.tensor.matmul(zr[:], w0c[:, 20 * q1:20 * (q1 + 1)],
                                 h1b[:, B * q1:B * (q1 + 1)],
                                 start=(q1 == 0), stop=False)
            nc.tensor.matmul(zr[:], w0g[0:64, :], gt0[0:64, :],
                             start=False, stop=True)

            z0p = sp.tile([20, B], F32)
            nc.vector.tensor_copy(z0p[:], zr[:])

            cc_in = dp.tile([20, B], F32)
            cc_out = dp.tile([20, B], F32, addr_space="Shared")
            nc.gpsimd.dma_start(out=cc_in[:], in_=z0p[:])
            nc.gpsimd.collective_compute(
                "AllReduce", ALU.add,
                replica_groups=[list(range(NCORES))],
                ins=[cc_in.opt()], outs=[cc_out.opt()])
            z0 = sp.tile([20, B], F32)
            nc.sync.dma_start(out=z0[:], in_=cc_out[:])

            # root BN: single bn_stats, mean/var from the 6-stat layout
            st0 = sp.tile([20, 6], F32)
            nc.vector.bn_stats(st0[:], z0[:])
            me0, mo0 = st0[:, 1:2], st0[:, 4:5]
            cve0, cvo0 = st0[:, 2:3], st0[:, 5:6]
            s0 = sp.tile([20, 1], F32)
            d0 = sp.tile([20, 1], F32)
            v0 = sp.tile([20, 1], F32)
            i0 = sp.tile([20, 1], F32)
            t0 = sp.tile([20, 1], F32)
            n0 = sp.tile([20, 1], F32)
            V = nc.vector
            V.tensor_tensor(s0[:], me0, mo0, op=ALU.add)
            V.tensor_tensor(d0[:], me0, mo0, op=ALU.subtract)
            V.tensor_tensor(v0[:], cve0, cvo0, op=ALU.add)
            V.tensor_tensor(t0[:], d0[:], d0[:], op=ALU.mult)
            V.tensor_scalar(v0[:], v0[:], 1.0 / B, None, op0=ALU.mult)
            V.tensor_scalar(t0[:], t0[:], 0.25, EPS, op0=ALU.mult, op1=ALU.add)
            V.tensor_tensor(v0[:], v0[:], t0[:], op=ALU.add)
            iv0 = i0[:].bitcast(mybir.dt.int32)
            V.tensor_scalar(iv0, v0[:].bitcast(mybir.dt.int32), 1, -1,
                            op0=ALU.arith_shift_right, op1=ALU.bitwise_xor)
            V.tensor_scalar(iv0, iv0, 0x5f3759e0, None, op0=ALU.add)
            for _ in range(2):
                V.tensor_tensor(n0[:], i0[:], i0[:], op=ALU.mult)
                V.tensor_tensor(n0[:], n0[:], v0[:], op=ALU.mult)
                V.tensor_scalar(n0[:], n0[:], -0.5, 1.5, op0=ALU.mult,
                                op1=ALU.add)
                V.tensor_tensor(i0[:], i0[:], n0[:], op=ALU.mult)
            sc0 = sp.tile([20, 1], F32)
            V.tensor_tensor(sc0[:], i0[:], g0c[:], op=ALU.mult)
            V.tensor_tensor(t0[:], s0[:], sc0[:], op=ALU.mult)
            V.tensor_scalar(t0[:], t0[:], 0.5, None, op0=ALU.mult)
            bi0 = sp.tile([20, 1], F32)
            V.tensor_tensor(bi0[:], be0c[:], t0[:], op=ALU.subtract)

            # h0 with a ones row at partition 32 so the bf16 head matmul
            # folds hb0 (rows 20..31 zeroed once).
            h0 = sp.tile([33, B], BF16)
            nc.vector.memset(h0[0:33, :], 0.0)
            nc.vector.memset(h0[32:33, :], 1.0)
            nc.scalar.activation(h0[0:20, :], z0[:], AF.Tanh,
                                 bias=bi0[:], scale=sc0[:])
            zh = zp.tile([1, B], F32, name="zh", tag="z")
            nc.tensor.matmul(zh[:], hw0hb[:], h0[:], start=True, stop=True)
            osb = sp.tile([1, B], F32)
            nc.vector.tensor_copy(osb[:], zh[:])
            nc.sync.dma_start(out=out_d[:], in_=osb[:])

    nc.compile()
    return nc


_PROGRAM = None


def _program():
    global _PROGRAM
    if _PROGRAM is None:
        _PROGRAM = _build_program()
    return _PROGRAM


# --------------------------------------------------------------------------
# host-side sharding / layout
# --------------------------------------------------------------------------

def _genes_pairs(genes_slice, group):
    """[B, T, G] fp32 -> pair tiles: [T//(2*group), 128, group*B] bf16.

    Pair p stacks term 2p's genes on K-rows 0-63 and term 2p+1's on 64-127.
    `group` pairs are packed per DMA tile."""
    t = genes_slice.shape[1]
    x = np.ascontiguousarray(genes_slice.transpose(1, 2, 0))      # [T, G, B]
    x = x.reshape(t // 2, 128, B)                                  # pairs
    p = t // 2
    x = x.reshape(p // group, group, 128, B).transpose(0, 2, 1, 3)
    return np.ascontiguousarray(x).reshape(p // group, 128, group * B) \
        .astype(_f8)


def _w_pairs(w_slice):
    """[L, 64, D] gene weights -> [128, (L/2)*64] bf16 block-diag pairs."""
    L = w_slice.shape[0]
    out = np.zeros((L // 2, 128, 64), np.float32)
    out[:, 0:64, 0:D] = w_slice[0::2]
    out[:, 64:128, 32:32 + D] = w_slice[1::2]
    out = out.transpose(1, 0, 2)
    return np.ascontiguousarray(out).reshape(128, (L // 2) * 64).astype(_bf16)


def _w_children(w_slice):
    """[L, 144, D] -> gappy [128, L*32] bf16 from children rows 0:80."""
    L = w_slice.shape[0]
    ch = w_slice[:, :80, :].reshape(L, 4, 20, D)
    out = np.zeros((L, 4, 32, 32), np.float32)
    out[:, :, :20, :D] = ch
    out = out.reshape(L, 128, 32).transpose(1, 0, 2)
    return np.ascontiguousarray(out).reshape(128, L * 32).astype(_bf16)


def _gappy_cols(vec_slice):
    """[L, D] -> [128, L/4] f32 with row 32j+d, col q = vec[4q+j, d]."""
    L = vec_slice.shape[0]
    arr = vec_slice.reshape(L // 4, 4, D)
    out = np.zeros((L // 4, 4, 32), np.float32)
    out[:, :, :D] = arr
    out = out.reshape(L // 4, 128).T
    return np.ascontiguousarray(out)


def _f32_to_bf2(a):
    """fp32 array -> byte-identical bf16 view with doubled last dim."""
    return np.ascontiguousarray(a.astype(np.float32)).view(_bf16)


def _prep_core(c, iv):
    s3 = slice(L3 * c, L3 * (c + 1))
    s2 = slice(L2 * c, L2 * (c + 1))
    s1 = slice(L1 * c, L1 * (c + 1))

    w0 = iv['W0'][0]                                    # [2624, 20]
    w0h = w0[:T1 * D, :].reshape(T1, D, D)[L1 * c:L1 * (c + 1)]   # [16, 20, 20]
    arr = w0h.reshape(Q1, 4, 20, D)
    w0c = np.zeros((Q1, 4, 32, D), np.float32)
    w0c[:, :, :20, :] = arr
    w0c = w0c.reshape(Q1, 128, D).transpose(1, 0, 2)
    w0c = np.ascontiguousarray(w0c).reshape(128, Q1 * D).astype(_bf16)

    hw0hb = np.zeros((33, 1), np.float32)
    hw0hb[:20, 0] = iv['hw0'][0][:, 0]
    hw0hb[32, 0] = iv['hb0'].reshape(-1)[0]

    w3p = _w_pairs(iv['W3'][s3])                        # [128, P3*64]
    w3ch = w3p.reshape(128, Q3 // CHUNK, 2 * CHUNK * 64).transpose(1, 0, 2)
    w3ch = np.ascontiguousarray(w3ch)

    gt0 = np.zeros((128, B), _f8)
    gt0[0:64, :] = iv['genes0'][:, 0, :].T.astype(_f8)
    gt0 = gt0.view(_bf16)
    w0g = np.zeros((128, 20), _bf16)
    w0g[0:64, :] = (w0[T1 * D:, :] / NCORES).astype(_bf16)

    def pad128(a20, rows):
        out = np.zeros((128, a20.shape[1]), np.float32)
        out[0:rows] = a20
        return out

    blob = np.zeros((128, BLOB_COLS), _bf16)

    def put(name, arr):
        a, b = _BL[name]
        assert arr.shape[1] == b - a, (name, arr.shape, b - a)
        blob[:, a:b] = arr

    put("w2c", _w_children(iv['W2'][s2]))
    put("w2g", _w_pairs(iv['W2'][s2][:, 80:144, :]))
    put("w1c", _w_children(iv['W1'][s1]))
    put("w1g", _w_pairs(iv['W1'][s1][:, 80:144, :]))
    put("w0c", w0c)
    put("gt1", _genes_pairs(iv['genes1'][:, s1, :], P1)[0].view(_bf16))
    put("gt0", gt0)
    put("w0g", w0g)
    put("g2b", _f32_to_bf2(_gappy_cols(iv['g2'][s2])))
    put("be2b", _f32_to_bf2(_gappy_cols(iv['be2'][s2])))
    put("g1b", _f32_to_bf2(_gappy_cols(iv['g1'][s1])))
    put("be1b", _f32_to_bf2(_gappy_cols(iv['be1'][s1])))
    put("g0c", _f32_to_bf2(pad128(iv['g0'].reshape(1, D).T, 20)))
    put("be0c", _f32_to_bf2(pad128(iv['be0'].reshape(1, D).T, 20)))
    hwb = np.zeros((128, 2), _bf16)
    hwb[0:33, 0:1] = pad128(hw0hb, 33)[0:33].astype(_bf16)
    put("hw0hb", hwb)

    return {
        'gt3': _genes_pairs(iv['genes3'][:, s3, :], 2 * CHUNK),
        'gt2': _genes_pairs(iv['genes2'][:, s2, :], 2 * CHUNK),
        'w3': w3ch,
        'g3b': _gappy_cols(iv['g3'][s3]),
        'be3b': _gappy_cols(iv['be3'][s3]),
        'blob': blob,
    }


def _prep_inputs(inputs):
    iv = {k: np.asarray(v, dtype=np.float32) for k, v in inputs.items()}
    return [_prep_core(c, iv) for c in range(NCORES)]


def run(in_maps, **kwargs):
    nc = _program()
    return run_bass_kernel_spmd(nc, in_maps, core_ids=list(range(NCORES)), **kwargs)


def kernel(**inputs) -> np.ndarray:
    in_maps = _prep_inputs(inputs)
    res = run(in_maps)
    pred = np.asarray(res.results[0]['out'], dtype=np.float32)   # [1, B]
    return np.ascontiguousarray(pred.T)                          # [B, 1]
